# revision 16
# baseline (speedup 1.0000x reference)
"""Trainium2 Bass kernel for nn_FNDE (FNO neural-ODE).

Mathematical collapse (validated to ~5e-7 rel err vs the f32 jax reference):
Each Fourier layer's RK4 integrates dz/dt = f(z) where f (the FNO spectral
conv) is affine, and *linear per rfft2 mode*: retained modes evolve through a
CxC complex matrix, non-retained modes are untouched, the bias only feeds mode
(0,0).  Lift (1->C) and projection (C->64->1) are pointwise linear.  Hence the
whole network is a per-mode complex scalar acting on rfft2(z):

    out[b] = s_base * z[b] + irfft2(S' * rfft2(z[b]))  + c_total

with S' supported on 33 x-rows (kx in [0..16] u [112..127]) x 16 y-cols.
The ky=0 column needs care: irfft2's implicit Hermitian projection couples the
two retained row blocks (top via W1, bottom via conj(W2)) each evaluation.

The device kernel evaluates the restricted DFT chain as TensorEngine matmuls
per sample (data-parallel: 2 samples per core across 8 cores):
    PT  = z.T @ FxC                  (x-DFT, re/im fused, output transposed)
    QQT = CyS.T @ PT                 (y-DFT cos/sin blocks)
    Q   = combine(QQT)               (complex assembly, transposed layout)
    W'  = S' (.) Q                   (elementwise spectral multiplier)
    V   = [W'r;W'i] via L1.T@CEy + L2.T@SEy   (y-inverse)
    O   = ICIS.T @ V + s_base * z    (x-inverse + identity path, PSUM accum)
    out = O + c_total
"""

import numpy as np

B, C, D, M, L = 16, 64, 128, 16, 3
NCORES = 8
SPC = B // NCORES  # samples per core
KXS = np.concatenate([np.arange(17), np.arange(112, 128)])  # 33 retained rows
NR = len(KXS)  # 33


# ----------------------------------------------------------------------------
# host-side weight folding (numpy, float64)
# ----------------------------------------------------------------------------

def _rk4(f, x, ts):
    for i in range(len(ts) - 1):
        h = ts[i + 1] - ts[i]
        k1 = f(x)
        k2 = f(x + 0.5 * h * k1)
        k3 = f(x + 0.5 * h * k2)
        k4 = f(x + h * k3)
        x = x + (h / 6.0) * (k1 + 2 * k2 + 2 * k3 + k4)
    return x


def _fold_weights(inputs):
    lift_w = np.asarray(inputs["lift_w"], np.float64)[:, 0]      # [C]
    lift_b = np.asarray(inputs["lift_b"], np.float64)            # [C]
    w1 = np.asarray(inputs["spec_w1"], np.float64)               # [L,C,C,M,M,2]
    w2 = np.asarray(inputs["spec_w2"], np.float64)
    fl_bias = np.asarray(inputs["fl_bias"], np.float64)          # [L,C]
    p1_w = np.asarray(inputs["p1_w"], np.float64)
    p1_b = np.asarray(inputs["p1_b"], np.float64)
    p2_w = np.asarray(inputs["p2_w"], np.float64)
    p2_b = np.asarray(inputs["p2_b"], np.float64)
    ts = np.asarray(inputs["samp_ts"], np.float64)

    q = (p2_w @ p1_w)[0]                                         # [C]
    c_proj = float((p2_w @ p1_b + p2_b)[0])
    s_base = float(q @ lift_w)

    w1c = w1[..., 0] + 1j * w1[..., 1]                           # [L,C(i),C(o),M,M]
    w2c = w2[..., 0] + 1j * w2[..., 1]
    # einsum "bixy,ioxy->boxy": out_o = sum_i v_i W[i,o]  =>  generator = W^T
    G1 = np.transpose(w1c, (0, 4, 3, 2, 1))                      # [L,ky,kx,o,i]
    G2 = np.transpose(w2c, (0, 4, 3, 2, 1))                      # [L,ky,r,o,i] rows 112+r

    qc = q.astype(np.complex128)

    def chain(G_stack):
        # propagate lift_w through L layers of 4-step RK4 (linear, no bias),
        # then project with q -> per-mode scalar
        x = np.broadcast_to(lift_w, G_stack.shape[1:-2] + (C,)).astype(np.complex128)
        for layer in range(L):
            Gl = G_stack[layer]
            x = _rk4(lambda v: np.einsum("...ij,...j->...i", Gl, v), x, ts)
        return np.einsum("c,...c->...", qc, x)

    s_top = chain(G1[:, 1:])                                     # [15(ky=1..15),16(kx)]
    s_bot = chain(G2[:, 1:])                                     # [15,16(r)]

    # ky = 0 column: Hermitian projection couples the blocks. Independent
    # coords m in [0,16]; generators per layer:
    G0 = np.zeros((L, 17, C, C), np.complex128)
    for layer in range(L):
        G0[layer, 0] = np.real(G1[layer, 0, 0])
        for m in range(1, 16):
            G0[layer, m] = 0.5 * (G1[layer, 0, m] + np.conj(G2[layer, 0, 16 - m]))
        G0[layer, 16] = 0.5 * np.conj(G2[layer, 0, 0])
    s_col0 = chain(G0)                                           # [17]

    # affine offset at mode (0,0): propagate lift_b*D^2 with layer biases
    y = (lift_b * D * D).astype(np.complex128)
    for layer in range(L):
        Gl = G0[layer, 0]
        bl = (fl_bias[layer] * D * D).astype(np.complex128)
        y = _rk4(lambda v: Gl @ v + bl, y, ts)
    t_const = float(np.real(qc @ y))
    c_total = c_proj + t_const / (D * D)

    # assemble restricted multiplier S' = S - s_base on [33 rows, 16 cols]
    Sp = np.full((NR, 16), s_base, np.complex128)
    Sp[0:16, 1:16] = s_top.T                                     # [kx, ky]
    Sp[17:33, 1:16] = s_bot.T
    Sp[0:17, 0] = s_col0
    for r in range(16):                                          # stored bottom of ky=0
        Sp[17 + r, 0] = np.conj(s_col0[16 - r])
    Sp = Sp - s_base

    # ------------------------------------------------------------------
    # constant matrices for the device matmul chain (float32)
    # ------------------------------------------------------------------
    xg = np.arange(D, dtype=np.float64)
    th_x = 2.0 * np.pi * np.outer(xg, KXS) / D                   # [x, r]
    fxc = np.concatenate([np.cos(th_x), -np.sin(th_x)], axis=1)  # [128, 66]

    ky = np.arange(16, dtype=np.float64)
    th_y = 2.0 * np.pi * np.outer(xg, ky) / D                    # [y, ky]
    cys = np.concatenate([np.cos(th_y), np.sin(th_y)], axis=1)   # [128, 32]

    cc = np.where(ky == 0, 1.0, 2.0) / D                         # [16]
    cey = cc[:, None] * np.cos(th_y.T)                           # [16, 128]
    sey = cc[:, None] * np.sin(th_y.T)

    icis = np.concatenate([np.cos(th_x.T), -np.sin(th_x.T)], axis=0) / D  # [66,128]

    srt = Sp.real.T.astype(np.float32)                           # [16, 33]
    sit = Sp.imag.T.astype(np.float32)

    return dict(
        fxc=fxc.astype(np.float32),
        cys=cys.astype(np.float32),
        cey=cey.astype(np.float32),
        sey=sey.astype(np.float32),
        icis=icis.astype(np.float32),
        srt4=np.tile(srt, (1, 2 * SPC)).astype(np.float32),      # [16, 132]
        sit4=np.tile(sit, (1, 2 * SPC)).astype(np.float32),
        s_base=np.float32(s_base),
        c_total=np.float32(c_total),
    )


# blob layouts shared by host packing and the device kernel
AW = 99            # blob A: [128, AW]
A_FXC, A_CYC, A_CYS, A_SB = 0, 66, 82, 98
BW = 904           # blob B: [67, BW]
B_ICIS, B_CEY, B_SEY, B_SRT, B_SIT, B_CROW = 0, 128, 256, 384, 516, 648


def _pack_blobs(k):
    ba = np.zeros((D, AW), np.float32)
    ba[:, A_FXC:A_FXC + 2 * NR] = k["fxc"]
    ba[:, A_CYC:A_CYC + 32] = k["cys"]
    ba[:, A_SB] = k["s_base"]
    bb = np.zeros((2 * NR + 1, BW), np.float32)
    bb[0:66, B_ICIS:B_ICIS + D] = k["icis"]
    bb[66, B_ICIS:B_ICIS + D] = 1.0        # ones row of the extended x-inverse
    bb[0:16, B_CEY:B_CEY + D] = k["cey"]
    bb[0:16, B_SEY:B_SEY + D] = k["sey"]
    bb[0:16, B_SRT:B_SRT + 2 * NR * SPC] = k["srt4"]
    bb[0:16, B_SIT:B_SIT + 2 * NR * SPC] = k["sit4"]
    bb[66, B_CROW:B_CROW + SPC * D] = k["c_total"]  # V row 66 -> + c_total
    return ba, bb


# ----------------------------------------------------------------------------
# numpy simulation of the exact device chain (for validation / fallback)
# ----------------------------------------------------------------------------

def _device_sim(z2, k):
    """z2: [SPC,128,128] f32; k: folded consts. Mirrors the Bass kernel."""
    out = np.empty_like(z2)
    for s in range(SPC):
        z = z2[s]
        pt = z.T @ k["fxc"]                                      # [y, 66]
        qc = k["cys"][:, 0:16].T @ pt                            # [16, 66]
        qs = k["cys"][:, 16:32].T @ pt
        qr = qc[:, 0:33] + qs[:, 33:66]                          # [ky, r]
        qi = qc[:, 33:66] - qs[:, 0:33]
        srt = k["srt4"][:, 0:NR]
        sit = k["sit4"][:, 0:NR]
        wr = srt * qr - sit * qi
        wi = srt * qi + sit * qr
        l1 = np.concatenate([wr, wi], axis=1)                    # [16, 66]
        l2 = np.concatenate([-wi, wr], axis=1)
        v = l1.T @ k["cey"] + l2.T @ k["sey"]                    # [66, 128]
        o = k["icis"].T @ v + k["s_base"] * z                    # [128, 128]
        out[s] = o + k["c_total"]
    return out


def _shard_inputs(z, consts):
    ba, bb = _pack_blobs(consts)
    return [
        {"zc": np.ascontiguousarray(z[i * SPC:(i + 1) * SPC, 0]),
         "ba": ba, "bb": bb}
        for i in range(NCORES)
    ]


# ----------------------------------------------------------------------------
# Bass kernel
# ----------------------------------------------------------------------------

_NC_CACHE = {}
LAST_RESULT = None  # BassKernelResults of the most recent device run


def _build_nc():
    import concourse.bacc as bacc
    import concourse.mybir as mybir
    import concourse.tile as tile

    f32 = mybir.dt.float32
    ALU = mybir.AluOpType
    nc = bacc.Bacc("TRN2", target_bir_lowering=False, debug=False,
                   num_devices=NCORES)
    zc = nc.dram_tensor("zc", [SPC, D, D], f32, kind="ExternalInput")
    ba = nc.dram_tensor("ba", [D, AW], f32, kind="ExternalInput")
    bb = nc.dram_tensor("bb", [2 * NR + 1, BW], f32, kind="ExternalInput")
    outc = nc.dram_tensor("outc", [SPC, D, D], f32, kind="ExternalOutput")

    W = 2 * NR  # 66

    with tile.TileContext(nc) as tc:
        with (
            tc.tile_pool(name="const", bufs=1) as cpool,
            tc.tile_pool(name="work", bufs=2) as wpool,
            tc.tile_pool(name="psum", bufs=1, space="PSUM") as ppool,
        ):
            # constants arrive as two packed blobs, issued from otherwise-idle
            # sequencers so the z DMA on sync isn't queued behind them
            t_ba = cpool.tile([D, AW], f32)
            t_bb = cpool.tile([W + 1, BW], f32)
            nc.scalar.dma_start(t_ba[:], ba[:])
            nc.scalar.dma_start(t_bb[:], bb[:])

            t_z = wpool.tile([D, SPC, D], f32)
            nc.sync.dma_start(t_z[:], zc.rearrange("s x y -> x s y"))

            # x-forward DFT (output already transposed): PT_s = z_s.T @ FxC
            pt_ps = ppool.tile([D, SPC, W], f32)
            for s in range(SPC):
                nc.tensor.matmul(pt_ps[:, s, :], t_z[:, s, :],
                                 t_ba[:, A_FXC:A_FXC + W], start=True, stop=True)
            t_ptb = wpool.tile([D, SPC, W], f32)
            nc.vector.tensor_copy(t_ptb[:], pt_ps[:])

            # y-forward DFT, cos and sin blocks both on partitions 0:16
            # (two-input DVE ops require equal base partitions)
            qq_ps = ppool.tile([16, 2, SPC, W], f32)
            nc.tensor.matmul(qq_ps[:, 0, :, :], t_ba[:, A_CYC:A_CYC + 16],
                             t_ptb[:], start=True, stop=True)
            nc.tensor.matmul(qq_ps[:, 1, :, :], t_ba[:, A_CYS:A_CYS + 16],
                             t_ptb[:], start=True, stop=True)
            t_qq = wpool.tile([16, 2, SPC, W], f32)
            nc.vector.tensor_copy(t_qq[:], qq_ps[:])

            # complex assembly (transposed layout [ky, r]), batched over samples
            t_qt = wpool.tile([16, SPC, 2, NR], f32)
            nc.vector.tensor_add(t_qt[:, :, 0, :],
                                 t_qq[:, 0, :, 0:NR], t_qq[:, 1, :, NR:W])
            nc.vector.tensor_sub(t_qt[:, :, 1, :],
                                 t_qq[:, 0, :, NR:W], t_qq[:, 1, :, 0:NR])

            # spectral multiplier: W'r = Sr*Qr - Si*Qi ; W'i = Sr*Qi + Si*Qr
            srt = t_bb[0:16, B_SRT:B_SRT + W * SPC].rearrange(
                "k (s b r) -> k s b r", s=SPC, b=2)
            sit = t_bb[0:16, B_SIT:B_SIT + W * SPC].rearrange(
                "k (s b r) -> k s b r", s=SPC, b=2)
            t_m1 = wpool.tile([16, SPC, 2, NR], f32)
            t_m2 = wpool.tile([16, SPC, 2, NR], f32)
            nc.vector.tensor_mul(t_m1[:], srt, t_qt[:])
            nc.vector.tensor_mul(t_m2[:], sit, t_qt[:])
            t_l1 = wpool.tile([16, SPC, 2, NR], f32)
            t_l2 = wpool.tile([16, SPC, 2, NR], f32)
            nc.vector.tensor_sub(t_l1[:, :, 0, :], t_m1[:, :, 0, :], t_m2[:, :, 1, :])
            nc.vector.tensor_add(t_l1[:, :, 1, :], t_m1[:, :, 1, :], t_m2[:, :, 0, :])
            nc.vector.tensor_scalar_mul(t_l2[:, :, 0, :], t_l1[:, :, 1, :], -1.0)
            nc.vector.tensor_copy(t_l2[:, :, 1, :], t_l1[:, :, 0, :])

            # y-inverse: V = [Vr; Vi] = L1.T @ CEy + L2.T @ SEy
            v_ps = ppool.tile([W, SPC, D], f32)
            for s in range(SPC):
                nc.tensor.matmul(v_ps[:, s, :], t_l1[:, s, :, :],
                                 t_bb[0:16, B_CEY:B_CEY + D], start=True, stop=False)
                nc.tensor.matmul(v_ps[:, s, :], t_l2[:, s, :, :],
                                 t_bb[0:16, B_SEY:B_SEY + D], start=False, stop=True)
            t_v = wpool.tile([W + 1, SPC, D], f32)
            # V row 66 = c_total; paired with the ones row 66 of the extended
            # x-inverse matrix this adds the scalar bias inside the matmul.
            # Engine partition bases must be 32-aligned, so copy rows 64:67
            # from the blob first and let the V copy overwrite rows 64:65.
            nc.vector.tensor_copy(t_v[64:W + 1, :, :],
                                  t_bb[64:W + 1, B_CROW:B_CROW + SPC * D])
            nc.vector.tensor_copy(t_v[0:W, :, :], v_ps[:])

            # x-inverse (incl. bias row), then out = s_base * z + O fused
            o_ps = ppool.tile([D, SPC, D], f32)
            for s in range(SPC):
                nc.tensor.matmul(o_ps[:, s, :], t_bb[:, B_ICIS:B_ICIS + D],
                                 t_v[:, s, :], start=True, stop=True)
            t_out = wpool.tile([D, SPC, D], f32)
            nc.vector.scalar_tensor_tensor(
                t_out[:], t_z[:], t_ba[:, A_SB:A_SB + 1], o_ps[:],
                op0=ALU.mult, op1=ALU.add)

            nc.sync.dma_start(outc.rearrange("s x y -> x s y"), t_out[:])

    nc.finalize()
    return nc


def _run_device(z, consts):
    global LAST_RESULT
    from concourse.bass_utils import run_bass_kernel_spmd

    if "nc" not in _NC_CACHE:
        _NC_CACHE["nc"] = _build_nc()
    nc = _NC_CACHE["nc"]
    in_maps = _shard_inputs(z, consts)
    res = run_bass_kernel_spmd(nc, in_maps, core_ids=list(range(NCORES)))
    LAST_RESULT = res
    out = np.empty((B, 1, D, D), np.float32)
    for i in range(NCORES):
        out[i * SPC:(i + 1) * SPC, 0] = res.results[i]["outc"]
    return out


def kernel(z, lift_w, lift_b, spec_w1, spec_w2, fl_bias, p1_w, p1_b, p2_w, p2_b,
           samp_ts):
    inputs = dict(z=z, lift_w=lift_w, lift_b=lift_b, spec_w1=spec_w1,
                  spec_w2=spec_w2, fl_bias=fl_bias, p1_w=p1_w, p1_b=p1_b,
                  p2_w=p2_w, p2_b=p2_b, samp_ts=samp_ts)
    consts = _fold_weights(inputs)
    z = np.asarray(z, np.float32)
    return _run_device(z, consts)


def kernel_numpy(z, **kw):
    """Pure-numpy path running the same folded math (validation only)."""
    inputs = dict(z=z, **kw)
    consts = _fold_weights(inputs)
    z = np.asarray(z, np.float32)
    out = np.empty((B, 1, D, D), np.float32)
    for i in range(NCORES):
        out[i * SPC:(i + 1) * SPC, 0] = _device_sim(z[i * SPC:(i + 1) * SPC, 0], consts)
    return out


# revision 17
# speedup vs baseline: 1.2760x; 1.2760x over previous
"""Trainium2 Bass kernel for nn_FNDE (FNO neural-ODE).

Mathematical collapse (validated to ~5e-7 rel err vs the f32 jax reference):
Each Fourier layer's RK4 integrates dz/dt = f(z) where f (the FNO spectral
conv) is affine, and *linear per rfft2 mode*: retained modes evolve through a
CxC complex matrix, non-retained modes are untouched, the bias only feeds mode
(0,0).  Lift (1->C) and projection (C->64->1) are pointwise linear.  Hence the
whole network is a per-mode complex scalar acting on rfft2(z):

    out[b] = s_base * z[b] + irfft2(S' * rfft2(z[b]))  + c_total

with S' supported on 33 x-rows (kx in [0..16] u [112..127]) x 16 y-cols.
The ky=0 column needs care: irfft2's implicit Hermitian projection couples the
two retained row blocks (top via W1, bottom via conj(W2)) each evaluation.

The device kernel evaluates the restricted DFT chain as TensorEngine matmuls
per sample (data-parallel: 2 samples per core across 8 cores):
    PT  = z.T @ FxC                  (x-DFT, re/im fused, output transposed)
    QQT = CyS.T @ PT                 (y-DFT cos/sin blocks)
    Q   = combine(QQT)               (complex assembly, transposed layout)
    W'  = S' (.) Q                   (elementwise spectral multiplier)
    V   = [W'r;W'i] via L1.T@CEy + L2.T@SEy   (y-inverse)
    O   = ICIS.T @ V + s_base * z    (x-inverse + identity path, PSUM accum)
    out = O + c_total
"""

import numpy as np

B, C, D, M, L = 16, 64, 128, 16, 3
NCORES = 8
SPC = B // NCORES  # samples per core
KXS = np.concatenate([np.arange(17), np.arange(112, 128)])  # 33 retained rows
NR = len(KXS)  # 33


# ----------------------------------------------------------------------------
# host-side weight folding (numpy, float64)
# ----------------------------------------------------------------------------

def _rk4(f, x, ts):
    for i in range(len(ts) - 1):
        h = ts[i + 1] - ts[i]
        k1 = f(x)
        k2 = f(x + 0.5 * h * k1)
        k3 = f(x + 0.5 * h * k2)
        k4 = f(x + h * k3)
        x = x + (h / 6.0) * (k1 + 2 * k2 + 2 * k3 + k4)
    return x


def _fold_weights(inputs):
    lift_w = np.asarray(inputs["lift_w"], np.float64)[:, 0]      # [C]
    lift_b = np.asarray(inputs["lift_b"], np.float64)            # [C]
    w1 = np.asarray(inputs["spec_w1"], np.float64)               # [L,C,C,M,M,2]
    w2 = np.asarray(inputs["spec_w2"], np.float64)
    fl_bias = np.asarray(inputs["fl_bias"], np.float64)          # [L,C]
    p1_w = np.asarray(inputs["p1_w"], np.float64)
    p1_b = np.asarray(inputs["p1_b"], np.float64)
    p2_w = np.asarray(inputs["p2_w"], np.float64)
    p2_b = np.asarray(inputs["p2_b"], np.float64)
    ts = np.asarray(inputs["samp_ts"], np.float64)

    q = (p2_w @ p1_w)[0]                                         # [C]
    c_proj = float((p2_w @ p1_b + p2_b)[0])
    s_base = float(q @ lift_w)

    w1c = w1[..., 0] + 1j * w1[..., 1]                           # [L,C(i),C(o),M,M]
    w2c = w2[..., 0] + 1j * w2[..., 1]
    # einsum "bixy,ioxy->boxy": out_o = sum_i v_i W[i,o]  =>  generator = W^T
    G1 = np.transpose(w1c, (0, 4, 3, 2, 1))                      # [L,ky,kx,o,i]
    G2 = np.transpose(w2c, (0, 4, 3, 2, 1))                      # [L,ky,r,o,i] rows 112+r

    qc = q.astype(np.complex128)

    def chain(G_stack):
        # propagate lift_w through L layers of 4-step RK4 (linear, no bias),
        # then project with q -> per-mode scalar
        x = np.broadcast_to(lift_w, G_stack.shape[1:-2] + (C,)).astype(np.complex128)
        for layer in range(L):
            Gl = G_stack[layer]
            x = _rk4(lambda v: np.einsum("...ij,...j->...i", Gl, v), x, ts)
        return np.einsum("c,...c->...", qc, x)

    s_top = chain(G1[:, 1:])                                     # [15(ky=1..15),16(kx)]
    s_bot = chain(G2[:, 1:])                                     # [15,16(r)]

    # ky = 0 column: Hermitian projection couples the blocks. Independent
    # coords m in [0,16]; generators per layer:
    G0 = np.zeros((L, 17, C, C), np.complex128)
    for layer in range(L):
        G0[layer, 0] = np.real(G1[layer, 0, 0])
        for m in range(1, 16):
            G0[layer, m] = 0.5 * (G1[layer, 0, m] + np.conj(G2[layer, 0, 16 - m]))
        G0[layer, 16] = 0.5 * np.conj(G2[layer, 0, 0])
    s_col0 = chain(G0)                                           # [17]

    # affine offset at mode (0,0): propagate lift_b*D^2 with layer biases
    y = (lift_b * D * D).astype(np.complex128)
    for layer in range(L):
        Gl = G0[layer, 0]
        bl = (fl_bias[layer] * D * D).astype(np.complex128)
        y = _rk4(lambda v: Gl @ v + bl, y, ts)
    t_const = float(np.real(qc @ y))
    c_total = c_proj + t_const / (D * D)

    # assemble restricted multiplier S' = S - s_base on [33 rows, 16 cols]
    Sp = np.full((NR, 16), s_base, np.complex128)
    Sp[0:16, 1:16] = s_top.T                                     # [kx, ky]
    Sp[17:33, 1:16] = s_bot.T
    Sp[0:17, 0] = s_col0
    for r in range(16):                                          # stored bottom of ky=0
        Sp[17 + r, 0] = np.conj(s_col0[16 - r])
    Sp = Sp - s_base

    # ------------------------------------------------------------------
    # constant matrices for the device matmul chain (float32)
    # ------------------------------------------------------------------
    xg = np.arange(D, dtype=np.float64)
    th_x = 2.0 * np.pi * np.outer(xg, KXS) / D                   # [x, r]
    fxc = np.concatenate([np.cos(th_x), -np.sin(th_x)], axis=1)  # [128, 66]

    ky = np.arange(16, dtype=np.float64)
    th_y = 2.0 * np.pi * np.outer(xg, ky) / D                    # [y, ky]
    cys = np.concatenate([np.cos(th_y), np.sin(th_y)], axis=1)   # [128, 32]

    cc = np.where(ky == 0, 1.0, 2.0) / D                         # [16]
    cey = cc[:, None] * np.cos(th_y.T)                           # [16, 128]
    sey = cc[:, None] * np.sin(th_y.T)

    icis = np.concatenate([np.cos(th_x.T), -np.sin(th_x.T)], axis=0) / D  # [66,128]

    srt = Sp.real.T.astype(np.float32)                           # [16, 33]
    sit = Sp.imag.T.astype(np.float32)

    return dict(
        fxc=fxc.astype(np.float32),
        cys=cys.astype(np.float32),
        cey=cey.astype(np.float32),
        sey=sey.astype(np.float32),
        icis=icis.astype(np.float32),
        srt4=np.tile(srt, (1, 2 * SPC)).astype(np.float32),      # [16, 132]
        sit4=np.tile(sit, (1, 2 * SPC)).astype(np.float32),
        s_base=np.float32(s_base),
        c_total=np.float32(c_total),
    )


# single constant blob [128, GW] — full 128 partitions so the HWDGE splits
# the transfer across all 16 DMA engines (odd partition counts degrade to a
# single-queue chunked transfer)
G_FXC, G_CYC, G_CYS, G_SB = 0, 66, 82, 98
G_ICIS, G_CEY, G_SEY, G_SRT, G_SIT, G_CROW = 99, 227, 355, 483, 615, 747
GW = 1003


def _pack_blobs(k):
    bl = np.zeros((D, GW), np.float32)
    bl[:, G_FXC:G_FXC + 2 * NR] = k["fxc"]
    bl[:, G_CYC:G_CYC + 32] = k["cys"]
    bl[:, G_SB] = k["s_base"]
    bl[0:66, G_ICIS:G_ICIS + D] = k["icis"]
    bl[66, G_ICIS:G_ICIS + D] = 1.0        # ones row of the extended x-inverse
    bl[0:16, G_CEY:G_CEY + D] = k["cey"]
    bl[0:16, G_SEY:G_SEY + D] = k["sey"]
    bl[0:16, G_SRT:G_SRT + 2 * NR * SPC] = k["srt4"]
    bl[0:16, G_SIT:G_SIT + 2 * NR * SPC] = k["sit4"]
    bl[66, G_CROW:G_CROW + SPC * D] = k["c_total"]  # V row 66 -> + c_total
    return bl


# ----------------------------------------------------------------------------
# numpy simulation of the exact device chain (for validation / fallback)
# ----------------------------------------------------------------------------

def _device_sim(z2, k):
    """z2: [SPC,128,128] f32; k: folded consts. Mirrors the Bass kernel."""
    out = np.empty_like(z2)
    for s in range(SPC):
        z = z2[s]
        pt = z.T @ k["fxc"]                                      # [y, 66]
        qc = k["cys"][:, 0:16].T @ pt                            # [16, 66]
        qs = k["cys"][:, 16:32].T @ pt
        qr = qc[:, 0:33] + qs[:, 33:66]                          # [ky, r]
        qi = qc[:, 33:66] - qs[:, 0:33]
        srt = k["srt4"][:, 0:NR]
        sit = k["sit4"][:, 0:NR]
        wr = srt * qr - sit * qi
        wi = srt * qi + sit * qr
        l1 = np.concatenate([wr, wi], axis=1)                    # [16, 66]
        l2 = np.concatenate([-wi, wr], axis=1)
        v = l1.T @ k["cey"] + l2.T @ k["sey"]                    # [66, 128]
        o = k["icis"].T @ v + k["s_base"] * z                    # [128, 128]
        out[s] = o + k["c_total"]
    return out


def _shard_inputs(z, consts):
    bl = _pack_blobs(consts)
    return [
        {"zc": np.ascontiguousarray(z[i * SPC:(i + 1) * SPC, 0]), "bl": bl}
        for i in range(NCORES)
    ]


# ----------------------------------------------------------------------------
# Bass kernel
# ----------------------------------------------------------------------------

_NC_CACHE = {}
LAST_RESULT = None  # BassKernelResults of the most recent device run


def _build_nc():
    import concourse.bacc as bacc
    import concourse.mybir as mybir
    import concourse.tile as tile

    f32 = mybir.dt.float32
    ALU = mybir.AluOpType
    nc = bacc.Bacc("TRN2", target_bir_lowering=False, debug=False,
                   num_devices=NCORES)
    zc = nc.dram_tensor("zc", [SPC, D, D], f32, kind="ExternalInput")
    bl = nc.dram_tensor("bl", [D, GW], f32, kind="ExternalInput")
    outc = nc.dram_tensor("outc", [SPC, D, D], f32, kind="ExternalOutput")

    W = 2 * NR  # 66

    with tile.TileContext(nc) as tc:
        with (
            tc.tile_pool(name="const", bufs=1) as cpool,
            tc.tile_pool(name="work", bufs=2) as wpool,
            tc.tile_pool(name="psum", bufs=1, space="PSUM") as ppool,
        ):
            # constants arrive as one packed blob, issued from the otherwise-
            # idle scalar sequencer so the z DMA on sync runs in parallel
            t_bl = cpool.tile([D, GW], f32)
            nc.scalar.dma_start(t_bl[:], bl[:])

            t_z = wpool.tile([D, SPC, D], f32)
            nc.sync.dma_start(t_z[:], zc.rearrange("s x y -> x s y"))

            # x-forward DFT (output already transposed): PT_s = z_s.T @ FxC
            pt_ps = ppool.tile([D, SPC, W], f32)
            for s in range(SPC):
                nc.tensor.matmul(pt_ps[:, s, :], t_z[:, s, :],
                                 t_bl[:, G_FXC:G_FXC + W], start=True, stop=True)
            t_ptb = wpool.tile([D, SPC, W], f32)
            nc.vector.tensor_copy(t_ptb[:], pt_ps[:])

            # y-forward DFT, cos and sin blocks both on partitions 0:16
            # (two-input DVE ops require equal base partitions)
            qq_ps = ppool.tile([16, 2, SPC, W], f32)
            nc.tensor.matmul(qq_ps[:, 0, :, :], t_bl[:, G_CYC:G_CYC + 16],
                             t_ptb[:], start=True, stop=True)
            nc.tensor.matmul(qq_ps[:, 1, :, :], t_bl[:, G_CYS:G_CYS + 16],
                             t_ptb[:], start=True, stop=True)
            t_qq = wpool.tile([16, 2, SPC, W], f32)
            nc.vector.tensor_copy(t_qq[:], qq_ps[:])

            # complex assembly (transposed layout [ky, r]), batched over samples
            t_qt = wpool.tile([16, SPC, 2, NR], f32)
            nc.vector.tensor_add(t_qt[:, :, 0, :],
                                 t_qq[:, 0, :, 0:NR], t_qq[:, 1, :, NR:W])
            nc.vector.tensor_sub(t_qt[:, :, 1, :],
                                 t_qq[:, 0, :, NR:W], t_qq[:, 1, :, 0:NR])

            # spectral multiplier: W'r = Sr*Qr - Si*Qi ; W'i = Sr*Qi + Si*Qr
            srt = t_bl[0:16, G_SRT:G_SRT + W * SPC].rearrange(
                "k (s b r) -> k s b r", s=SPC, b=2)
            sit = t_bl[0:16, G_SIT:G_SIT + W * SPC].rearrange(
                "k (s b r) -> k s b r", s=SPC, b=2)
            t_m1 = wpool.tile([16, SPC, 2, NR], f32)
            t_m2 = wpool.tile([16, SPC, 2, NR], f32)
            nc.vector.tensor_mul(t_m1[:], srt, t_qt[:])
            nc.vector.tensor_mul(t_m2[:], sit, t_qt[:])
            t_l1 = wpool.tile([16, SPC, 2, NR], f32)
            t_l2 = wpool.tile([16, SPC, 2, NR], f32)
            nc.vector.tensor_sub(t_l1[:, :, 0, :], t_m1[:, :, 0, :], t_m2[:, :, 1, :])
            nc.vector.tensor_add(t_l1[:, :, 1, :], t_m1[:, :, 1, :], t_m2[:, :, 0, :])
            nc.vector.tensor_scalar_mul(t_l2[:, :, 0, :], t_l1[:, :, 1, :], -1.0)
            nc.vector.tensor_copy(t_l2[:, :, 1, :], t_l1[:, :, 0, :])

            # y-inverse: V = [Vr; Vi] = L1.T @ CEy + L2.T @ SEy
            v_ps = ppool.tile([W, SPC, D], f32)
            for s in range(SPC):
                nc.tensor.matmul(v_ps[:, s, :], t_l1[:, s, :, :],
                                 t_bl[0:16, G_CEY:G_CEY + D], start=True, stop=False)
                nc.tensor.matmul(v_ps[:, s, :], t_l2[:, s, :, :],
                                 t_bl[0:16, G_SEY:G_SEY + D], start=False, stop=True)
            t_v = wpool.tile([W + 1, SPC, D], f32)
            # V row 66 = c_total; paired with the ones row 66 of the extended
            # x-inverse matrix this adds the scalar bias inside the matmul.
            # Engine partition bases must be 32-aligned, so copy rows 64:67
            # from the blob first and let the V copy overwrite rows 64:65.
            nc.vector.tensor_copy(t_v[64:W + 1, :, :],
                                  t_bl[64:W + 1, G_CROW:G_CROW + SPC * D])
            nc.vector.tensor_copy(t_v[0:W, :, :], v_ps[:])

            # x-inverse (incl. bias row), then out = s_base * z + O fused
            o_ps = ppool.tile([D, SPC, D], f32)
            for s in range(SPC):
                nc.tensor.matmul(o_ps[:, s, :], t_bl[0:W + 1, G_ICIS:G_ICIS + D],
                                 t_v[:, s, :], start=True, stop=True)
            t_out = wpool.tile([D, SPC, D], f32)
            nc.vector.scalar_tensor_tensor(
                t_out[:], t_z[:], t_bl[:, G_SB:G_SB + 1], o_ps[:],
                op0=ALU.mult, op1=ALU.add)

            nc.sync.dma_start(outc.rearrange("s x y -> x s y"), t_out[:])

    nc.finalize()
    return nc


def _run_device(z, consts):
    global LAST_RESULT
    from concourse.bass_utils import run_bass_kernel_spmd

    if "nc" not in _NC_CACHE:
        _NC_CACHE["nc"] = _build_nc()
    nc = _NC_CACHE["nc"]
    in_maps = _shard_inputs(z, consts)
    res = run_bass_kernel_spmd(nc, in_maps, core_ids=list(range(NCORES)))
    LAST_RESULT = res
    out = np.empty((B, 1, D, D), np.float32)
    for i in range(NCORES):
        out[i * SPC:(i + 1) * SPC, 0] = res.results[i]["outc"]
    return out


def kernel(z, lift_w, lift_b, spec_w1, spec_w2, fl_bias, p1_w, p1_b, p2_w, p2_b,
           samp_ts):
    inputs = dict(z=z, lift_w=lift_w, lift_b=lift_b, spec_w1=spec_w1,
                  spec_w2=spec_w2, fl_bias=fl_bias, p1_w=p1_w, p1_b=p1_b,
                  p2_w=p2_w, p2_b=p2_b, samp_ts=samp_ts)
    consts = _fold_weights(inputs)
    z = np.asarray(z, np.float32)
    return _run_device(z, consts)


def kernel_numpy(z, **kw):
    """Pure-numpy path running the same folded math (validation only)."""
    inputs = dict(z=z, **kw)
    consts = _fold_weights(inputs)
    z = np.asarray(z, np.float32)
    out = np.empty((B, 1, D, D), np.float32)
    for i in range(NCORES):
        out[i * SPC:(i + 1) * SPC, 0] = _device_sim(z[i * SPC:(i + 1) * SPC, 0], consts)
    return out


# revision 20
# speedup vs baseline: 1.3491x; 1.0573x over previous
"""Trainium2 Bass kernel for nn_FNDE (FNO neural-ODE).

Mathematical collapse (validated to ~5e-7 rel err vs the f32 jax reference):
Each Fourier layer's RK4 integrates dz/dt = f(z) where f (the FNO spectral
conv) is affine, and *linear per rfft2 mode*: retained modes evolve through a
CxC complex matrix, non-retained modes are untouched, the bias only feeds mode
(0,0).  Lift (1->C) and projection (C->64->1) are pointwise linear.  Hence the
whole network is a per-mode complex scalar acting on rfft2(z):

    out[b] = s_base * z[b] + irfft2(S' * rfft2(z[b]))  + c_total

with S' supported on 33 x-rows (kx in [0..16] u [112..127]) x 16 y-cols.
The ky=0 column needs care: irfft2's implicit Hermitian projection couples the
two retained row blocks (top via W1, bottom via conj(W2)) each evaluation.

The device kernel evaluates the restricted DFT chain as TensorEngine matmuls
per sample (data-parallel: 2 samples per core across 8 cores):
    PT  = z.T @ FxC                  (x-DFT, re/im fused, output transposed)
    QQT = CyS.T @ PT                 (y-DFT cos/sin blocks)
    Q   = combine(QQT)               (complex assembly, transposed layout)
    W'  = S' (.) Q                   (elementwise spectral multiplier)
    V   = [W'r;W'i] via L1.T@CEy + L2.T@SEy   (y-inverse)
    O   = ICIS.T @ V + s_base * z    (x-inverse + identity path, PSUM accum)
    out = O + c_total
"""

import numpy as np

B, C, D, M, L = 16, 64, 128, 16, 3
NCORES = 8
SPC = B // NCORES  # samples per core
KXS = np.concatenate([np.arange(17), np.arange(112, 128)])  # 33 retained rows
NR = len(KXS)  # 33


# ----------------------------------------------------------------------------
# host-side weight folding (numpy, float64)
# ----------------------------------------------------------------------------

def _rk4(f, x, ts):
    for i in range(len(ts) - 1):
        h = ts[i + 1] - ts[i]
        k1 = f(x)
        k2 = f(x + 0.5 * h * k1)
        k3 = f(x + 0.5 * h * k2)
        k4 = f(x + h * k3)
        x = x + (h / 6.0) * (k1 + 2 * k2 + 2 * k3 + k4)
    return x


def _fold_weights(inputs):
    lift_w = np.asarray(inputs["lift_w"], np.float64)[:, 0]      # [C]
    lift_b = np.asarray(inputs["lift_b"], np.float64)            # [C]
    w1 = np.asarray(inputs["spec_w1"], np.float64)               # [L,C,C,M,M,2]
    w2 = np.asarray(inputs["spec_w2"], np.float64)
    fl_bias = np.asarray(inputs["fl_bias"], np.float64)          # [L,C]
    p1_w = np.asarray(inputs["p1_w"], np.float64)
    p1_b = np.asarray(inputs["p1_b"], np.float64)
    p2_w = np.asarray(inputs["p2_w"], np.float64)
    p2_b = np.asarray(inputs["p2_b"], np.float64)
    ts = np.asarray(inputs["samp_ts"], np.float64)

    q = (p2_w @ p1_w)[0]                                         # [C]
    c_proj = float((p2_w @ p1_b + p2_b)[0])
    s_base = float(q @ lift_w)

    w1c = w1[..., 0] + 1j * w1[..., 1]                           # [L,C(i),C(o),M,M]
    w2c = w2[..., 0] + 1j * w2[..., 1]
    # einsum "bixy,ioxy->boxy": out_o = sum_i v_i W[i,o]  =>  generator = W^T
    G1 = np.transpose(w1c, (0, 4, 3, 2, 1))                      # [L,ky,kx,o,i]
    G2 = np.transpose(w2c, (0, 4, 3, 2, 1))                      # [L,ky,r,o,i] rows 112+r

    qc = q.astype(np.complex128)

    def chain(G_stack):
        # propagate lift_w through L layers of 4-step RK4 (linear, no bias),
        # then project with q -> per-mode scalar
        x = np.broadcast_to(lift_w, G_stack.shape[1:-2] + (C,)).astype(np.complex128)
        for layer in range(L):
            Gl = G_stack[layer]
            x = _rk4(lambda v: np.einsum("...ij,...j->...i", Gl, v), x, ts)
        return np.einsum("c,...c->...", qc, x)

    s_top = chain(G1[:, 1:])                                     # [15(ky=1..15),16(kx)]
    s_bot = chain(G2[:, 1:])                                     # [15,16(r)]

    # ky = 0 column: Hermitian projection couples the blocks. Independent
    # coords m in [0,16]; generators per layer:
    G0 = np.zeros((L, 17, C, C), np.complex128)
    for layer in range(L):
        G0[layer, 0] = np.real(G1[layer, 0, 0])
        for m in range(1, 16):
            G0[layer, m] = 0.5 * (G1[layer, 0, m] + np.conj(G2[layer, 0, 16 - m]))
        G0[layer, 16] = 0.5 * np.conj(G2[layer, 0, 0])
    s_col0 = chain(G0)                                           # [17]

    # affine offset at mode (0,0): propagate lift_b*D^2 with layer biases
    y = (lift_b * D * D).astype(np.complex128)
    for layer in range(L):
        Gl = G0[layer, 0]
        bl = (fl_bias[layer] * D * D).astype(np.complex128)
        y = _rk4(lambda v: Gl @ v + bl, y, ts)
    t_const = float(np.real(qc @ y))
    c_total = c_proj + t_const / (D * D)

    # assemble restricted multiplier S' = S - s_base on [33 rows, 16 cols]
    Sp = np.full((NR, 16), s_base, np.complex128)
    Sp[0:16, 1:16] = s_top.T                                     # [kx, ky]
    Sp[17:33, 1:16] = s_bot.T
    Sp[0:17, 0] = s_col0
    for r in range(16):                                          # stored bottom of ky=0
        Sp[17 + r, 0] = np.conj(s_col0[16 - r])
    Sp = Sp - s_base

    # ------------------------------------------------------------------
    # constant matrices for the device matmul chain (float32)
    # ------------------------------------------------------------------
    xg = np.arange(D, dtype=np.float64)
    th_x = 2.0 * np.pi * np.outer(xg, KXS) / D                   # [x, r]
    fxc = np.concatenate([np.cos(th_x), -np.sin(th_x)], axis=1)  # [128, 66]

    ky = np.arange(16, dtype=np.float64)
    th_y = 2.0 * np.pi * np.outer(xg, ky) / D                    # [y, ky]
    cys = np.concatenate([np.cos(th_y), np.sin(th_y)], axis=1)   # [128, 32]

    cc = np.where(ky == 0, 1.0, 2.0) / D                         # [16]
    cey = cc[:, None] * np.cos(th_y.T)                           # [16, 128]
    sey = cc[:, None] * np.sin(th_y.T)

    icis = np.concatenate([np.cos(th_x.T), -np.sin(th_x.T)], axis=0) / D  # [66,128]

    srt = Sp.real.T.astype(np.float32)                           # [16, 33]
    sit = Sp.imag.T.astype(np.float32)

    return dict(
        fxc=fxc.astype(np.float32),
        cys=cys.astype(np.float32),
        cey=cey.astype(np.float32),
        sey=sey.astype(np.float32),
        icis=icis.astype(np.float32),
        srt4=np.tile(srt, (1, 2 * SPC)).astype(np.float32),      # [16, 132]
        sit4=np.tile(sit, (1, 2 * SPC)).astype(np.float32),
        s_base=np.float32(s_base),
        c_total=np.float32(c_total),
    )


# single constant blob [128, GW] — full 128 partitions so the HWDGE splits
# the transfer across all 16 DMA engines (odd partition counts degrade to a
# single-queue chunked transfer)
G_FXC, G_CYC, G_CYS, G_SB = 0, 66, 82, 98
G_ICIS, G_CEY, G_SEY, G_SRT, G_SIT, G_CROW = 99, 227, 355, 483, 615, 747
GW = 1003


def _pack_blobs(k):
    bl = np.zeros((D, GW), np.float32)
    bl[:, G_FXC:G_FXC + 2 * NR] = k["fxc"]
    bl[:, G_CYC:G_CYC + 32] = k["cys"]
    bl[:, G_SB] = k["s_base"]
    bl[0:66, G_ICIS:G_ICIS + D] = k["icis"]
    bl[66, G_ICIS:G_ICIS + D] = 1.0        # ones row of the extended x-inverse
    bl[0:16, G_CEY:G_CEY + D] = k["cey"]
    bl[0:16, G_SEY:G_SEY + D] = k["sey"]
    bl[0:16, G_SRT:G_SRT + 2 * NR * SPC] = k["srt4"]
    bl[0:16, G_SIT:G_SIT + 2 * NR * SPC] = k["sit4"]
    bl[66, G_CROW:G_CROW + SPC * D] = k["c_total"]  # V row 66 -> + c_total
    return bl


# ----------------------------------------------------------------------------
# numpy simulation of the exact device chain (for validation / fallback)
# ----------------------------------------------------------------------------

def _device_sim(z2, k):
    """z2: [SPC,128,128] f32; k: folded consts. Mirrors the Bass kernel."""
    out = np.empty_like(z2)
    for s in range(SPC):
        z = z2[s]
        pt = z.T @ k["fxc"]                                      # [y, 66]
        qc = k["cys"][:, 0:16].T @ pt                            # [16, 66]
        qs = k["cys"][:, 16:32].T @ pt
        qr = qc[:, 0:33] + qs[:, 33:66]                          # [ky, r]
        qi = qc[:, 33:66] - qs[:, 0:33]
        srt = k["srt4"][:, 0:NR]
        sit = k["sit4"][:, 0:NR]
        wr = srt * qr - sit * qi
        wi = srt * qi + sit * qr
        l1 = np.concatenate([wr, wi], axis=1)                    # [16, 66]
        l2 = np.concatenate([-wi, wr], axis=1)
        v = l1.T @ k["cey"] + l2.T @ k["sey"]                    # [66, 128]
        o = k["icis"].T @ v + k["s_base"] * z                    # [128, 128]
        out[s] = o + k["c_total"]
    return out


def _shard_inputs(z, consts):
    bl = _pack_blobs(consts)
    return [
        {"zc": np.ascontiguousarray(z[i * SPC:(i + 1) * SPC, 0]), "bl": bl}
        for i in range(NCORES)
    ]




# ----------------------------------------------------------------------------
# raw (non-Tile) Bass kernel: hand-scheduled semaphores, no Tile tail barrier
# ----------------------------------------------------------------------------

# blob 1 [128, B1W]: needed before MM1/MM2.  blob 2 [128, B2W]: needed later.
B1W = 99
R_FXC, R_CYC, R_CYS, R_SB = 0, 66, 82, 98
B2W = 904
R_ICIS, R_CEY, R_SEY, R_SRT, R_SIT, R_CROW = 0, 128, 256, 384, 516, 648


def _pack_blobs_raw(k):
    b1 = np.zeros((D, B1W), np.float32)
    b1[:, R_FXC:R_FXC + 2 * NR] = k["fxc"]
    b1[:, R_CYC:R_CYC + 32] = k["cys"]
    b1[:, R_SB] = k["s_base"]
    b2 = np.zeros((D, B2W), np.float32)
    b2[0:66, R_ICIS:R_ICIS + D] = k["icis"]
    b2[66, R_ICIS:R_ICIS + D] = 1.0
    b2[0:16, R_CEY:R_CEY + D] = k["cey"]
    b2[0:16, R_SEY:R_SEY + D] = k["sey"]
    b2[0:16, R_SRT:R_SRT + 2 * NR * SPC] = k["srt4"]
    b2[0:16, R_SIT:R_SIT + 2 * NR * SPC] = k["sit4"]
    b2[66, R_CROW:R_CROW + SPC * D] = k["c_total"]
    return b1, b2


def _build_nc_raw():
    import concourse.bacc as bacc
    import concourse.mybir as mybir

    f32 = mybir.dt.float32
    ALU = mybir.AluOpType
    nc = bacc.Bacc("TRN2", target_bir_lowering=False, debug=False,
                   num_devices=NCORES)
    zc = nc.dram_tensor("zc", [SPC, D, D], f32, kind="ExternalInput")
    b1 = nc.dram_tensor("b1", [D, B1W], f32, kind="ExternalInput")
    b2 = nc.dram_tensor("b2", [D, B2W], f32, kind="ExternalInput")
    outc = nc.dram_tensor("outc", [SPC, D, D], f32, kind="ExternalOutput")

    W = 2 * NR  # 66

    t_b1 = nc.alloc_sbuf_tensor("t_b1", [D, B1W], f32).ap()
    t_b2 = nc.alloc_sbuf_tensor("t_b2", [D, B2W], f32).ap()
    t_z = nc.alloc_sbuf_tensor("t_z", [D, SPC, D], f32).ap()
    t_ptb = nc.alloc_sbuf_tensor("t_ptb", [D, SPC, W], f32).ap()
    t_qq = nc.alloc_sbuf_tensor("t_qq", [16, 2, SPC, W], f32).ap()
    t_qt = nc.alloc_sbuf_tensor("t_qt", [16, SPC, 2, NR], f32).ap()
    t_m1 = nc.alloc_sbuf_tensor("t_m1", [16, SPC, 2, NR], f32).ap()
    t_m2 = nc.alloc_sbuf_tensor("t_m2", [16, SPC, 2, NR], f32).ap()
    t_l1 = nc.alloc_sbuf_tensor("t_l1", [16, SPC, 2, NR], f32).ap()
    t_l2 = nc.alloc_sbuf_tensor("t_l2", [16, SPC, 2, NR], f32).ap()
    t_v = nc.alloc_sbuf_tensor("t_v", [W + 1, SPC, D], f32).ap()
    t_out = nc.alloc_sbuf_tensor("t_out", [D, SPC, D], f32).ap()
    pt_ps = nc.alloc_psum_tensor("pt_ps", [D, SPC, W], f32).ap()
    qq_ps = nc.alloc_psum_tensor("qq_ps", [16, 2, SPC, W], f32).ap()
    v_ps = nc.alloc_psum_tensor("v_ps", [W, SPC, D], f32).ap()
    o_ps = nc.alloc_psum_tensor("o_ps", [D, SPC, D], f32).ap()

    srt = t_b2[0:16, R_SRT:R_SRT + W * SPC].rearrange(
        "k (s b r) -> k s b r", s=SPC, b=2)
    sit = t_b2[0:16, R_SIT:R_SIT + W * SPC].rearrange(
        "k (s b r) -> k s b r", s=SPC, b=2)

    with (
        nc.semaphore("s_z") as s_z,
        nc.semaphore("s_b1") as s_b1,
        nc.semaphore("s_b2") as s_b2,
        nc.semaphore("s_pe") as s_pe,
        nc.semaphore("s_dve") as s_dve,
        nc.semaphore("s_fin") as s_fin,
        nc.semaphore("s_od") as s_od,
        nc.Block() as block,
    ):
        @block.sync
        def _(sync):
            sync.dma_start(t_z, zc.rearrange("s x y -> x s y")).then_inc(s_z, 16)
            sync.wait_ge(s_fin, 1)
            sync.dma_start(outc.rearrange("s x y -> x s y"), t_out).then_inc(
                s_od, 16)
            sync.wait_ge(s_od, 16)

        @block.scalar
        def _(scalar):
            scalar.dma_start(t_b1, b1[:]).then_inc(s_b1, 16)
            scalar.dma_start(t_b2, b2[:]).then_inc(s_b2, 16)

        @block.tensor
        def _(tensor):
            tensor.wait_ge(s_z, 16)
            tensor.wait_ge(s_b1, 16)
            for s in range(SPC):
                tensor.matmul(pt_ps[:, s, :], t_z[:, s, :],
                              t_b1[:, R_FXC:R_FXC + W],
                              start=True, stop=True).then_inc(s_pe, 1)
            tensor.wait_ge(s_dve, 1)
            tensor.matmul(qq_ps[:, 0, :, :], t_b1[:, R_CYC:R_CYC + 16],
                          t_ptb, start=True, stop=True).then_inc(s_pe, 1)
            tensor.matmul(qq_ps[:, 1, :, :], t_b1[:, R_CYS:R_CYS + 16],
                          t_ptb, start=True, stop=True).then_inc(s_pe, 1)
            tensor.wait_ge(s_dve, 10)        # l1/l2 ready (cey/sey via s_dve)
            for s in range(SPC):
                tensor.matmul(v_ps[:, s, :], t_l1[:, s, :, :],
                              t_b2[0:16, R_CEY:R_CEY + D],
                              start=True, stop=False).then_inc(s_pe, 1)
                tensor.matmul(v_ps[:, s, :], t_l2[:, s, :, :],
                              t_b2[0:16, R_SEY:R_SEY + D],
                              start=False, stop=True).then_inc(s_pe, 1)
            tensor.wait_ge(s_dve, 12)        # t_v assembled
            for s in range(SPC):
                tensor.matmul(o_ps[:, s, :], t_b2[0:W + 1, R_ICIS:R_ICIS + D],
                              t_v[:, s, :],
                              start=True, stop=True).then_inc(s_pe, 1)

        @block.vector
        def _(vector):
            # NB: the DVE pipeline overlaps its own instructions, so
            # same-engine RAW/WAW also needs explicit s_dve waits.
            vector.wait_ge(s_pe, 2)
            vector.tensor_copy(t_ptb, pt_ps).then_inc(s_dve, 1)        # 1
            vector.wait_ge(s_pe, 4)
            vector.tensor_copy(t_qq, qq_ps).then_inc(s_dve, 1)         # 2
            vector.wait_ge(s_dve, 2)
            vector.tensor_add(t_qt[:, :, 0, :], t_qq[:, 0, :, 0:NR],
                              t_qq[:, 1, :, NR:W]).then_inc(s_dve, 1)  # 3
            vector.tensor_sub(t_qt[:, :, 1, :], t_qq[:, 0, :, NR:W],
                              t_qq[:, 1, :, 0:NR]).then_inc(s_dve, 1)  # 4
            vector.wait_ge(s_b2, 16)
            vector.wait_ge(s_dve, 4)
            vector.tensor_mul(t_m1, srt, t_qt).then_inc(s_dve, 1)      # 5
            vector.tensor_mul(t_m2, sit, t_qt).then_inc(s_dve, 1)      # 6
            vector.wait_ge(s_dve, 6)
            vector.tensor_sub(t_l1[:, :, 0, :], t_m1[:, :, 0, :],
                              t_m2[:, :, 1, :]).then_inc(s_dve, 1)     # 7
            vector.tensor_add(t_l1[:, :, 1, :], t_m1[:, :, 1, :],
                              t_m2[:, :, 0, :]).then_inc(s_dve, 1)     # 8
            vector.wait_ge(s_dve, 8)
            vector.tensor_scalar_mul(t_l2[:, :, 0, :], t_l1[:, :, 1, :],
                                     -1.0).then_inc(s_dve, 1)          # 9
            vector.tensor_copy(t_l2[:, :, 1, :],
                               t_l1[:, :, 0, :]).then_inc(s_dve, 1)    # 10
            vector.tensor_copy(t_v[64:W + 1, :, :],
                               t_b2[64:W + 1, R_CROW:R_CROW + SPC * D]
                               ).then_inc(s_dve, 1)                    # 11
            vector.wait_ge(s_pe, 8)
            vector.wait_ge(s_dve, 11)
            vector.tensor_copy(t_v[0:W, :, :], v_ps).then_inc(s_dve, 1)  # 12
            vector.wait_ge(s_pe, 10)
            vector.scalar_tensor_tensor(t_out, t_z, t_b1[:, R_SB:R_SB + 1],
                                        o_ps, op0=ALU.mult,
                                        op1=ALU.add).then_inc(s_fin, 1)

    nc.finalize()
    return nc


def _shard_inputs_raw(z, consts):
    b1, b2 = _pack_blobs_raw(consts)
    return [
        {"zc": np.ascontiguousarray(z[i * SPC:(i + 1) * SPC, 0]),
         "b1": b1, "b2": b2}
        for i in range(NCORES)
    ]


# ----------------------------------------------------------------------------
# Bass kernel
# ----------------------------------------------------------------------------

_NC_CACHE = {}
LAST_RESULT = None  # BassKernelResults of the most recent device run


def _build_nc():
    import concourse.bacc as bacc
    import concourse.mybir as mybir
    import concourse.tile as tile

    f32 = mybir.dt.float32
    ALU = mybir.AluOpType
    nc = bacc.Bacc("TRN2", target_bir_lowering=False, debug=False,
                   num_devices=NCORES)
    zc = nc.dram_tensor("zc", [SPC, D, D], f32, kind="ExternalInput")
    bl = nc.dram_tensor("bl", [D, GW], f32, kind="ExternalInput")
    outc = nc.dram_tensor("outc", [SPC, D, D], f32, kind="ExternalOutput")

    W = 2 * NR  # 66

    with tile.TileContext(nc) as tc:
        with (
            tc.tile_pool(name="const", bufs=1) as cpool,
            tc.tile_pool(name="work", bufs=2) as wpool,
            tc.tile_pool(name="psum", bufs=1, space="PSUM") as ppool,
        ):
            # constants arrive as one packed blob, issued from the otherwise-
            # idle scalar sequencer so the z DMA on sync runs in parallel
            t_bl = cpool.tile([D, GW], f32)
            nc.scalar.dma_start(t_bl[:], bl[:])

            t_z = wpool.tile([D, SPC, D], f32)
            nc.sync.dma_start(t_z[:], zc.rearrange("s x y -> x s y"))

            # x-forward DFT (output already transposed): PT_s = z_s.T @ FxC
            pt_ps = ppool.tile([D, SPC, W], f32)
            for s in range(SPC):
                nc.tensor.matmul(pt_ps[:, s, :], t_z[:, s, :],
                                 t_bl[:, G_FXC:G_FXC + W], start=True, stop=True)
            t_ptb = wpool.tile([D, SPC, W], f32)
            nc.vector.tensor_copy(t_ptb[:], pt_ps[:])

            # y-forward DFT, cos and sin blocks both on partitions 0:16
            # (two-input DVE ops require equal base partitions)
            qq_ps = ppool.tile([16, 2, SPC, W], f32)
            nc.tensor.matmul(qq_ps[:, 0, :, :], t_bl[:, G_CYC:G_CYC + 16],
                             t_ptb[:], start=True, stop=True)
            nc.tensor.matmul(qq_ps[:, 1, :, :], t_bl[:, G_CYS:G_CYS + 16],
                             t_ptb[:], start=True, stop=True)
            t_qq = wpool.tile([16, 2, SPC, W], f32)
            nc.vector.tensor_copy(t_qq[:], qq_ps[:])

            # complex assembly (transposed layout [ky, r]), batched over samples
            t_qt = wpool.tile([16, SPC, 2, NR], f32)
            nc.vector.tensor_add(t_qt[:, :, 0, :],
                                 t_qq[:, 0, :, 0:NR], t_qq[:, 1, :, NR:W])
            nc.vector.tensor_sub(t_qt[:, :, 1, :],
                                 t_qq[:, 0, :, NR:W], t_qq[:, 1, :, 0:NR])

            # spectral multiplier: W'r = Sr*Qr - Si*Qi ; W'i = Sr*Qi + Si*Qr
            srt = t_bl[0:16, G_SRT:G_SRT + W * SPC].rearrange(
                "k (s b r) -> k s b r", s=SPC, b=2)
            sit = t_bl[0:16, G_SIT:G_SIT + W * SPC].rearrange(
                "k (s b r) -> k s b r", s=SPC, b=2)
            t_m1 = wpool.tile([16, SPC, 2, NR], f32)
            t_m2 = wpool.tile([16, SPC, 2, NR], f32)
            nc.vector.tensor_mul(t_m1[:], srt, t_qt[:])
            nc.vector.tensor_mul(t_m2[:], sit, t_qt[:])
            t_l1 = wpool.tile([16, SPC, 2, NR], f32)
            t_l2 = wpool.tile([16, SPC, 2, NR], f32)
            nc.vector.tensor_sub(t_l1[:, :, 0, :], t_m1[:, :, 0, :], t_m2[:, :, 1, :])
            nc.vector.tensor_add(t_l1[:, :, 1, :], t_m1[:, :, 1, :], t_m2[:, :, 0, :])
            nc.vector.tensor_scalar_mul(t_l2[:, :, 0, :], t_l1[:, :, 1, :], -1.0)
            nc.vector.tensor_copy(t_l2[:, :, 1, :], t_l1[:, :, 0, :])

            # y-inverse: V = [Vr; Vi] = L1.T @ CEy + L2.T @ SEy
            v_ps = ppool.tile([W, SPC, D], f32)
            for s in range(SPC):
                nc.tensor.matmul(v_ps[:, s, :], t_l1[:, s, :, :],
                                 t_bl[0:16, G_CEY:G_CEY + D], start=True, stop=False)
                nc.tensor.matmul(v_ps[:, s, :], t_l2[:, s, :, :],
                                 t_bl[0:16, G_SEY:G_SEY + D], start=False, stop=True)
            t_v = wpool.tile([W + 1, SPC, D], f32)
            # V row 66 = c_total; paired with the ones row 66 of the extended
            # x-inverse matrix this adds the scalar bias inside the matmul.
            # Engine partition bases must be 32-aligned, so copy rows 64:67
            # from the blob first and let the V copy overwrite rows 64:65.
            nc.vector.tensor_copy(t_v[64:W + 1, :, :],
                                  t_bl[64:W + 1, G_CROW:G_CROW + SPC * D])
            nc.vector.tensor_copy(t_v[0:W, :, :], v_ps[:])

            # x-inverse (incl. bias row), then out = s_base * z + O fused
            o_ps = ppool.tile([D, SPC, D], f32)
            for s in range(SPC):
                nc.tensor.matmul(o_ps[:, s, :], t_bl[0:W + 1, G_ICIS:G_ICIS + D],
                                 t_v[:, s, :], start=True, stop=True)
            t_out = wpool.tile([D, SPC, D], f32)
            nc.vector.scalar_tensor_tensor(
                t_out[:], t_z[:], t_bl[:, G_SB:G_SB + 1], o_ps[:],
                op0=ALU.mult, op1=ALU.add)

            nc.sync.dma_start(outc.rearrange("s x y -> x s y"), t_out[:])

    nc.finalize()
    return nc


def _run_device(z, consts):
    global LAST_RESULT
    import os
    from concourse.bass_utils import run_bass_kernel_spmd

    raw = os.environ.get("FNDE_IMPL", "raw") == "raw"
    key = "raw" if raw else "tile"
    if key not in _NC_CACHE:
        _NC_CACHE[key] = _build_nc_raw() if raw else _build_nc()
    nc = _NC_CACHE[key]
    in_maps = _shard_inputs_raw(z, consts) if raw else _shard_inputs(z, consts)
    res = run_bass_kernel_spmd(nc, in_maps, core_ids=list(range(NCORES)))
    LAST_RESULT = res
    out = np.empty((B, 1, D, D), np.float32)
    for i in range(NCORES):
        out[i * SPC:(i + 1) * SPC, 0] = res.results[i]["outc"]
    return out


def kernel(z, lift_w, lift_b, spec_w1, spec_w2, fl_bias, p1_w, p1_b, p2_w, p2_b,
           samp_ts):
    inputs = dict(z=z, lift_w=lift_w, lift_b=lift_b, spec_w1=spec_w1,
                  spec_w2=spec_w2, fl_bias=fl_bias, p1_w=p1_w, p1_b=p1_b,
                  p2_w=p2_w, p2_b=p2_b, samp_ts=samp_ts)
    consts = _fold_weights(inputs)
    z = np.asarray(z, np.float32)
    return _run_device(z, consts)


def kernel_numpy(z, **kw):
    """Pure-numpy path running the same folded math (validation only)."""
    inputs = dict(z=z, **kw)
    consts = _fold_weights(inputs)
    z = np.asarray(z, np.float32)
    out = np.empty((B, 1, D, D), np.float32)
    for i in range(NCORES):
        out[i * SPC:(i + 1) * SPC, 0] = _device_sim(z[i * SPC:(i + 1) * SPC, 0], consts)
    return out


# revision 21
# speedup vs baseline: 1.5440x; 1.1445x over previous
"""Trainium2 Bass kernel for nn_FNDE (FNO neural-ODE).

Mathematical collapse (validated to ~5e-7 rel err vs the f32 jax reference):
Each Fourier layer's RK4 integrates dz/dt = f(z) where f (the FNO spectral
conv) is affine, and *linear per rfft2 mode*: retained modes evolve through a
CxC complex matrix, non-retained modes are untouched, the bias only feeds mode
(0,0).  Lift (1->C) and projection (C->64->1) are pointwise linear.  Hence the
whole network is a per-mode complex scalar acting on rfft2(z):

    out[b] = s_base * z[b] + irfft2(S' * rfft2(z[b]))  + c_total

with S' supported on 33 x-rows (kx in [0..16] u [112..127]) x 16 y-cols.
The ky=0 column needs care: irfft2's implicit Hermitian projection couples the
two retained row blocks (top via W1, bottom via conj(W2)) each evaluation.

The device kernel evaluates the restricted DFT chain as TensorEngine matmuls
per sample (data-parallel: 2 samples per core across 8 cores):
    PT  = z.T @ FxC                  (x-DFT, re/im fused, output transposed)
    QQT = CyS.T @ PT                 (y-DFT cos/sin blocks)
    Q   = combine(QQT)               (complex assembly, transposed layout)
    W'  = S' (.) Q                   (elementwise spectral multiplier)
    V   = [W'r;W'i] via L1.T@CEy + L2.T@SEy   (y-inverse)
    O   = ICIS.T @ V + s_base * z    (x-inverse + identity path, PSUM accum)
    out = O + c_total
"""

import numpy as np

B, C, D, M, L = 16, 64, 128, 16, 3
NCORES = 8
SPC = B // NCORES  # samples per core
KXS = np.concatenate([np.arange(17), np.arange(112, 128)])  # 33 retained rows
NR = len(KXS)  # 33


# ----------------------------------------------------------------------------
# host-side weight folding (numpy, float64)
# ----------------------------------------------------------------------------

def _rk4(f, x, ts):
    for i in range(len(ts) - 1):
        h = ts[i + 1] - ts[i]
        k1 = f(x)
        k2 = f(x + 0.5 * h * k1)
        k3 = f(x + 0.5 * h * k2)
        k4 = f(x + h * k3)
        x = x + (h / 6.0) * (k1 + 2 * k2 + 2 * k3 + k4)
    return x


def _fold_weights(inputs):
    lift_w = np.asarray(inputs["lift_w"], np.float64)[:, 0]      # [C]
    lift_b = np.asarray(inputs["lift_b"], np.float64)            # [C]
    w1 = np.asarray(inputs["spec_w1"], np.float64)               # [L,C,C,M,M,2]
    w2 = np.asarray(inputs["spec_w2"], np.float64)
    fl_bias = np.asarray(inputs["fl_bias"], np.float64)          # [L,C]
    p1_w = np.asarray(inputs["p1_w"], np.float64)
    p1_b = np.asarray(inputs["p1_b"], np.float64)
    p2_w = np.asarray(inputs["p2_w"], np.float64)
    p2_b = np.asarray(inputs["p2_b"], np.float64)
    ts = np.asarray(inputs["samp_ts"], np.float64)

    q = (p2_w @ p1_w)[0]                                         # [C]
    c_proj = float((p2_w @ p1_b + p2_b)[0])
    s_base = float(q @ lift_w)

    w1c = w1[..., 0] + 1j * w1[..., 1]                           # [L,C(i),C(o),M,M]
    w2c = w2[..., 0] + 1j * w2[..., 1]
    # einsum "bixy,ioxy->boxy": out_o = sum_i v_i W[i,o]  =>  generator = W^T
    G1 = np.transpose(w1c, (0, 4, 3, 2, 1))                      # [L,ky,kx,o,i]
    G2 = np.transpose(w2c, (0, 4, 3, 2, 1))                      # [L,ky,r,o,i] rows 112+r

    qc = q.astype(np.complex128)

    def chain(G_stack):
        # propagate lift_w through L layers of 4-step RK4 (linear, no bias),
        # then project with q -> per-mode scalar
        x = np.broadcast_to(lift_w, G_stack.shape[1:-2] + (C,)).astype(np.complex128)
        for layer in range(L):
            Gl = G_stack[layer]
            x = _rk4(lambda v: np.einsum("...ij,...j->...i", Gl, v), x, ts)
        return np.einsum("c,...c->...", qc, x)

    s_top = chain(G1[:, 1:])                                     # [15(ky=1..15),16(kx)]
    s_bot = chain(G2[:, 1:])                                     # [15,16(r)]

    # ky = 0 column: Hermitian projection couples the blocks. Independent
    # coords m in [0,16]; generators per layer:
    G0 = np.zeros((L, 17, C, C), np.complex128)
    for layer in range(L):
        G0[layer, 0] = np.real(G1[layer, 0, 0])
        for m in range(1, 16):
            G0[layer, m] = 0.5 * (G1[layer, 0, m] + np.conj(G2[layer, 0, 16 - m]))
        G0[layer, 16] = 0.5 * np.conj(G2[layer, 0, 0])
    s_col0 = chain(G0)                                           # [17]

    # affine offset at mode (0,0): propagate lift_b*D^2 with layer biases
    y = (lift_b * D * D).astype(np.complex128)
    for layer in range(L):
        Gl = G0[layer, 0]
        bl = (fl_bias[layer] * D * D).astype(np.complex128)
        y = _rk4(lambda v: Gl @ v + bl, y, ts)
    t_const = float(np.real(qc @ y))
    c_total = c_proj + t_const / (D * D)

    # assemble restricted multiplier S' = S - s_base on [33 rows, 16 cols]
    Sp = np.full((NR, 16), s_base, np.complex128)
    Sp[0:16, 1:16] = s_top.T                                     # [kx, ky]
    Sp[17:33, 1:16] = s_bot.T
    Sp[0:17, 0] = s_col0
    for r in range(16):                                          # stored bottom of ky=0
        Sp[17 + r, 0] = np.conj(s_col0[16 - r])
    Sp = Sp - s_base

    # ------------------------------------------------------------------
    # constant matrices for the device matmul chain (float32)
    # ------------------------------------------------------------------
    xg = np.arange(D, dtype=np.float64)
    th_x = 2.0 * np.pi * np.outer(xg, KXS) / D                   # [x, r]
    fxc = np.concatenate([np.cos(th_x), -np.sin(th_x)], axis=1)  # [128, 66]

    ky = np.arange(16, dtype=np.float64)
    th_y = 2.0 * np.pi * np.outer(xg, ky) / D                    # [y, ky]
    cys = np.concatenate([np.cos(th_y), np.sin(th_y)], axis=1)   # [128, 32]

    cc = np.where(ky == 0, 1.0, 2.0) / D                         # [16]
    cey = cc[:, None] * np.cos(th_y.T)                           # [16, 128]
    sey = cc[:, None] * np.sin(th_y.T)

    icis = np.concatenate([np.cos(th_x.T), -np.sin(th_x.T)], axis=0) / D  # [66,128]

    srt = Sp.real.T.astype(np.float32)                           # [16, 33]
    sit = Sp.imag.T.astype(np.float32)

    return dict(
        fxc=fxc.astype(np.float32),
        cys=cys.astype(np.float32),
        cey=cey.astype(np.float32),
        sey=sey.astype(np.float32),
        icis=icis.astype(np.float32),
        srt4=np.tile(srt, (1, 2 * SPC)).astype(np.float32),      # [16, 132]
        sit4=np.tile(sit, (1, 2 * SPC)).astype(np.float32),
        s_base=np.float32(s_base),
        c_total=np.float32(c_total),
    )


# single constant blob [128, GW] — full 128 partitions so the HWDGE splits
# the transfer across all 16 DMA engines (odd partition counts degrade to a
# single-queue chunked transfer)
G_FXC, G_CYC, G_CYS, G_SB = 0, 66, 82, 98
G_ICIS, G_CEY, G_SEY, G_SRT, G_SIT, G_CROW = 99, 227, 355, 483, 615, 747
GW = 1003


def _pack_blobs(k):
    bl = np.zeros((D, GW), np.float32)
    bl[:, G_FXC:G_FXC + 2 * NR] = k["fxc"]
    bl[:, G_CYC:G_CYC + 32] = k["cys"]
    bl[:, G_SB] = k["s_base"]
    bl[0:66, G_ICIS:G_ICIS + D] = k["icis"]
    bl[66, G_ICIS:G_ICIS + D] = 1.0        # ones row of the extended x-inverse
    bl[0:16, G_CEY:G_CEY + D] = k["cey"]
    bl[0:16, G_SEY:G_SEY + D] = k["sey"]
    bl[0:16, G_SRT:G_SRT + 2 * NR * SPC] = k["srt4"]
    bl[0:16, G_SIT:G_SIT + 2 * NR * SPC] = k["sit4"]
    bl[66, G_CROW:G_CROW + SPC * D] = k["c_total"]  # V row 66 -> + c_total
    return bl


# ----------------------------------------------------------------------------
# numpy simulation of the exact device chain (for validation / fallback)
# ----------------------------------------------------------------------------

def _device_sim(z2, k):
    """z2: [SPC,128,128] f32; k: folded consts. Mirrors the Bass kernel."""
    out = np.empty_like(z2)
    for s in range(SPC):
        z = z2[s]
        pt = z.T @ k["fxc"]                                      # [y, 66]
        qc = k["cys"][:, 0:16].T @ pt                            # [16, 66]
        qs = k["cys"][:, 16:32].T @ pt
        qr = qc[:, 0:33] + qs[:, 33:66]                          # [ky, r]
        qi = qc[:, 33:66] - qs[:, 0:33]
        srt = k["srt4"][:, 0:NR]
        sit = k["sit4"][:, 0:NR]
        wr = srt * qr - sit * qi
        wi = srt * qi + sit * qr
        l1 = np.concatenate([wr, wi], axis=1)                    # [16, 66]
        l2 = np.concatenate([-wi, wr], axis=1)
        v = l1.T @ k["cey"] + l2.T @ k["sey"]                    # [66, 128]
        o = k["icis"].T @ v + k["s_base"] * z                    # [128, 128]
        out[s] = o + k["c_total"]
    return out


def _shard_inputs(z, consts):
    bl = _pack_blobs(consts)
    return [
        {"zc": np.ascontiguousarray(z[i * SPC:(i + 1) * SPC, 0]), "bl": bl}
        for i in range(NCORES)
    ]




# ----------------------------------------------------------------------------
# raw (non-Tile) Bass kernel: hand-scheduled semaphores, no Tile tail barrier
# ----------------------------------------------------------------------------

# blob 1 (fp16) [128, B1W]: DFT matrices.  blob 2 (fp32) [128, B2W]: spectral
# multipliers (kept fp32 for the DVE stages), bias row and s_base column.
B1W = 482
R_FXC, R_CYC, R_CYS, R_ICIS, R_CEY, R_SEY = 0, 66, 82, 98, 226, 354
B2W = 521
R_SRT, R_SIT, R_CROW, R_SB = 0, 132, 264, 520


def _pack_blobs_raw(k):
    b1 = np.zeros((D, B1W), np.float16)
    b1[:, R_FXC:R_FXC + 2 * NR] = k["fxc"].astype(np.float16)
    b1[:, R_CYC:R_CYC + 32] = k["cys"].astype(np.float16)
    b1[0:66, R_ICIS:R_ICIS + D] = k["icis"].astype(np.float16)
    b1[66, R_ICIS:R_ICIS + D] = 1.0
    b1[0:16, R_CEY:R_CEY + D] = k["cey"].astype(np.float16)
    b1[0:16, R_SEY:R_SEY + D] = k["sey"].astype(np.float16)
    b2 = np.zeros((D, B2W), np.float32)
    b2[0:16, R_SRT:R_SRT + 2 * NR * SPC] = k["srt4"]
    b2[0:16, R_SIT:R_SIT + 2 * NR * SPC] = k["sit4"]
    b2[66, R_CROW:R_CROW + SPC * D] = k["c_total"]
    b2[:, R_SB] = k["s_base"]
    return b1, b2


def _build_nc_raw():
    import concourse.bacc as bacc
    import concourse.mybir as mybir

    f32 = mybir.dt.float32
    f16 = mybir.dt.float16
    ALU = mybir.AluOpType
    nc = bacc.Bacc("TRN2", target_bir_lowering=False, debug=False,
                   num_devices=NCORES)
    zc = nc.dram_tensor("zc", [SPC, D, D], f32, kind="ExternalInput")
    b1 = nc.dram_tensor("b1", [D, B1W], f16, kind="ExternalInput")
    b2 = nc.dram_tensor("b2", [D, B2W], f32, kind="ExternalInput")
    outc = nc.dram_tensor("outc", [SPC, D, D], f32, kind="ExternalOutput")

    W = 2 * NR  # 66

    t_b1 = nc.alloc_sbuf_tensor("t_b1", [D, B1W], f16).ap()
    t_b2 = nc.alloc_sbuf_tensor("t_b2", [D, B2W], f32).ap()
    t_z = nc.alloc_sbuf_tensor("t_z", [D, SPC, D], f32).ap()
    t_z16 = nc.alloc_sbuf_tensor("t_z16", [D, SPC, D], f16).ap()
    t_ptb = nc.alloc_sbuf_tensor("t_ptb", [D, SPC, W], f16).ap()
    t_qq = nc.alloc_sbuf_tensor("t_qq", [16, 2, SPC, W], f32).ap()
    t_qt = nc.alloc_sbuf_tensor("t_qt", [16, SPC, 2, NR], f32).ap()
    t_m1 = nc.alloc_sbuf_tensor("t_m1", [16, SPC, 2, NR], f32).ap()
    t_m2 = nc.alloc_sbuf_tensor("t_m2", [16, SPC, 2, NR], f32).ap()
    t_l1 = nc.alloc_sbuf_tensor("t_l1", [16, SPC, 2, NR], f16).ap()
    t_l2 = nc.alloc_sbuf_tensor("t_l2", [16, SPC, 2, NR], f16).ap()
    t_v = nc.alloc_sbuf_tensor("t_v", [W + 1, SPC, D], f16).ap()
    t_out = nc.alloc_sbuf_tensor("t_out", [D, SPC, D], f32).ap()
    pt_ps = nc.alloc_psum_tensor("pt_ps", [D, SPC, W], f32).ap()
    qq_ps = nc.alloc_psum_tensor("qq_ps", [16, 2, SPC, W], f32).ap()
    v_ps = nc.alloc_psum_tensor("v_ps", [W, SPC, D], f32).ap()
    o_ps = nc.alloc_psum_tensor("o_ps", [D, SPC, D], f32).ap()

    srt = t_b2[0:16, R_SRT:R_SRT + W * SPC].rearrange(
        "k (s b r) -> k s b r", s=SPC, b=2)
    sit = t_b2[0:16, R_SIT:R_SIT + W * SPC].rearrange(
        "k (s b r) -> k s b r", s=SPC, b=2)

    with (
        nc.semaphore("s_z") as s_z,
        nc.semaphore("s_b1") as s_b1,
        nc.semaphore("s_b2") as s_b2,
        nc.semaphore("s_pe") as s_pe,
        nc.semaphore("s_dve") as s_dve,
        nc.semaphore("s_fin") as s_fin,
        nc.semaphore("s_od") as s_od,
        nc.Block() as block,
    ):
        @block.sync
        def _(sync):
            sync.dma_start(t_z, zc.rearrange("s x y -> x s y")).then_inc(s_z, 16)
            sync.wait_ge(s_fin, 1)
            sync.dma_start(outc.rearrange("s x y -> x s y"), t_out).then_inc(
                s_od, 16)
            sync.wait_ge(s_od, 16)

        @block.scalar
        def _(scalar):
            scalar.dma_start(t_b1, b1[:]).then_inc(s_b1, 16)
            scalar.dma_start(t_b2, b2[:]).then_inc(s_b2, 16)

        @block.tensor
        def _(tensor):
            tensor.wait_ge(s_b1, 16)
            tensor.wait_ge(s_dve, 1)         # z16 cast
            for s in range(SPC):
                tensor.matmul(pt_ps[:, s, :], t_z16[:, s, :],
                              t_b1[:, R_FXC:R_FXC + W],
                              start=True, stop=True).then_inc(s_pe, 1)
            tensor.wait_ge(s_dve, 2)         # ptb ready
            tensor.matmul(qq_ps[:, 0, :, :], t_b1[:, R_CYC:R_CYC + 16],
                          t_ptb, start=True, stop=True).then_inc(s_pe, 1)
            tensor.matmul(qq_ps[:, 1, :, :], t_b1[:, R_CYS:R_CYS + 16],
                          t_ptb, start=True, stop=True).then_inc(s_pe, 1)
            tensor.wait_ge(s_dve, 11)        # l1/l2 ready
            for s in range(SPC):
                tensor.matmul(v_ps[:, s, :], t_l1[:, s, :, :],
                              t_b1[0:16, R_CEY:R_CEY + D],
                              start=True, stop=False).then_inc(s_pe, 1)
                tensor.matmul(v_ps[:, s, :], t_l2[:, s, :, :],
                              t_b1[0:16, R_SEY:R_SEY + D],
                              start=False, stop=True).then_inc(s_pe, 1)
            tensor.wait_ge(s_dve, 13)        # t_v assembled
            for s in range(SPC):
                tensor.matmul(o_ps[:, s, :], t_b1[0:W + 1, R_ICIS:R_ICIS + D],
                              t_v[:, s, :],
                              start=True, stop=True).then_inc(s_pe, 1)

        @block.vector
        def _(vector):
            # NB: the DVE pipeline overlaps its own instructions, so
            # same-engine RAW/WAW also needs explicit s_dve waits.
            vector.wait_ge(s_z, 16)
            vector.tensor_copy(t_z16, t_z).then_inc(s_dve, 1)          # 1
            vector.wait_ge(s_pe, 2)
            vector.tensor_copy(t_ptb, pt_ps).then_inc(s_dve, 1)        # 2
            vector.wait_ge(s_pe, 4)
            vector.tensor_copy(t_qq, qq_ps).then_inc(s_dve, 1)         # 3
            vector.wait_ge(s_dve, 3)
            vector.tensor_add(t_qt[:, :, 0, :], t_qq[:, 0, :, 0:NR],
                              t_qq[:, 1, :, NR:W]).then_inc(s_dve, 1)  # 4
            vector.tensor_sub(t_qt[:, :, 1, :], t_qq[:, 0, :, NR:W],
                              t_qq[:, 1, :, 0:NR]).then_inc(s_dve, 1)  # 5
            vector.wait_ge(s_b2, 16)
            vector.wait_ge(s_dve, 5)
            vector.tensor_mul(t_m1, srt, t_qt).then_inc(s_dve, 1)      # 6
            vector.tensor_mul(t_m2, sit, t_qt).then_inc(s_dve, 1)      # 7
            vector.wait_ge(s_dve, 7)
            vector.tensor_sub(t_l1[:, :, 0, :], t_m1[:, :, 0, :],
                              t_m2[:, :, 1, :]).then_inc(s_dve, 1)     # 8
            vector.tensor_add(t_l1[:, :, 1, :], t_m1[:, :, 1, :],
                              t_m2[:, :, 0, :]).then_inc(s_dve, 1)     # 9
            vector.wait_ge(s_dve, 9)
            vector.tensor_scalar_mul(t_l2[:, :, 0, :], t_l1[:, :, 1, :],
                                     -1.0).then_inc(s_dve, 1)          # 10
            vector.tensor_copy(t_l2[:, :, 1, :],
                               t_l1[:, :, 0, :]).then_inc(s_dve, 1)    # 11
            vector.tensor_copy(t_v[64:W + 1, :, :],
                               t_b2[64:W + 1, R_CROW:R_CROW + SPC * D]
                               ).then_inc(s_dve, 1)                    # 12
            vector.wait_ge(s_pe, 8)
            vector.wait_ge(s_dve, 12)
            vector.tensor_copy(t_v[0:W, :, :], v_ps).then_inc(s_dve, 1)  # 13
            vector.wait_ge(s_pe, 10)
            vector.scalar_tensor_tensor(t_out, t_z, t_b2[:, R_SB:R_SB + 1],
                                        o_ps, op0=ALU.mult,
                                        op1=ALU.add).then_inc(s_fin, 1)

    nc.finalize()
    return nc


def _shard_inputs_raw(z, consts):
    b1, b2 = _pack_blobs_raw(consts)
    return [
        {"zc": np.ascontiguousarray(z[i * SPC:(i + 1) * SPC, 0]),
         "b1": b1, "b2": b2}
        for i in range(NCORES)
    ]


# ----------------------------------------------------------------------------
# Bass kernel
# ----------------------------------------------------------------------------

_NC_CACHE = {}
LAST_RESULT = None  # BassKernelResults of the most recent device run


def _build_nc():
    import concourse.bacc as bacc
    import concourse.mybir as mybir
    import concourse.tile as tile

    f32 = mybir.dt.float32
    ALU = mybir.AluOpType
    nc = bacc.Bacc("TRN2", target_bir_lowering=False, debug=False,
                   num_devices=NCORES)
    zc = nc.dram_tensor("zc", [SPC, D, D], f32, kind="ExternalInput")
    bl = nc.dram_tensor("bl", [D, GW], f32, kind="ExternalInput")
    outc = nc.dram_tensor("outc", [SPC, D, D], f32, kind="ExternalOutput")

    W = 2 * NR  # 66

    with tile.TileContext(nc) as tc:
        with (
            tc.tile_pool(name="const", bufs=1) as cpool,
            tc.tile_pool(name="work", bufs=2) as wpool,
            tc.tile_pool(name="psum", bufs=1, space="PSUM") as ppool,
        ):
            # constants arrive as one packed blob, issued from the otherwise-
            # idle scalar sequencer so the z DMA on sync runs in parallel
            t_bl = cpool.tile([D, GW], f32)
            nc.scalar.dma_start(t_bl[:], bl[:])

            t_z = wpool.tile([D, SPC, D], f32)
            nc.sync.dma_start(t_z[:], zc.rearrange("s x y -> x s y"))

            # x-forward DFT (output already transposed): PT_s = z_s.T @ FxC
            pt_ps = ppool.tile([D, SPC, W], f32)
            for s in range(SPC):
                nc.tensor.matmul(pt_ps[:, s, :], t_z[:, s, :],
                                 t_bl[:, G_FXC:G_FXC + W], start=True, stop=True)
            t_ptb = wpool.tile([D, SPC, W], f32)
            nc.vector.tensor_copy(t_ptb[:], pt_ps[:])

            # y-forward DFT, cos and sin blocks both on partitions 0:16
            # (two-input DVE ops require equal base partitions)
            qq_ps = ppool.tile([16, 2, SPC, W], f32)
            nc.tensor.matmul(qq_ps[:, 0, :, :], t_bl[:, G_CYC:G_CYC + 16],
                             t_ptb[:], start=True, stop=True)
            nc.tensor.matmul(qq_ps[:, 1, :, :], t_bl[:, G_CYS:G_CYS + 16],
                             t_ptb[:], start=True, stop=True)
            t_qq = wpool.tile([16, 2, SPC, W], f32)
            nc.vector.tensor_copy(t_qq[:], qq_ps[:])

            # complex assembly (transposed layout [ky, r]), batched over samples
            t_qt = wpool.tile([16, SPC, 2, NR], f32)
            nc.vector.tensor_add(t_qt[:, :, 0, :],
                                 t_qq[:, 0, :, 0:NR], t_qq[:, 1, :, NR:W])
            nc.vector.tensor_sub(t_qt[:, :, 1, :],
                                 t_qq[:, 0, :, NR:W], t_qq[:, 1, :, 0:NR])

            # spectral multiplier: W'r = Sr*Qr - Si*Qi ; W'i = Sr*Qi + Si*Qr
            srt = t_bl[0:16, G_SRT:G_SRT + W * SPC].rearrange(
                "k (s b r) -> k s b r", s=SPC, b=2)
            sit = t_bl[0:16, G_SIT:G_SIT + W * SPC].rearrange(
                "k (s b r) -> k s b r", s=SPC, b=2)
            t_m1 = wpool.tile([16, SPC, 2, NR], f32)
            t_m2 = wpool.tile([16, SPC, 2, NR], f32)
            nc.vector.tensor_mul(t_m1[:], srt, t_qt[:])
            nc.vector.tensor_mul(t_m2[:], sit, t_qt[:])
            t_l1 = wpool.tile([16, SPC, 2, NR], f32)
            t_l2 = wpool.tile([16, SPC, 2, NR], f32)
            nc.vector.tensor_sub(t_l1[:, :, 0, :], t_m1[:, :, 0, :], t_m2[:, :, 1, :])
            nc.vector.tensor_add(t_l1[:, :, 1, :], t_m1[:, :, 1, :], t_m2[:, :, 0, :])
            nc.vector.tensor_scalar_mul(t_l2[:, :, 0, :], t_l1[:, :, 1, :], -1.0)
            nc.vector.tensor_copy(t_l2[:, :, 1, :], t_l1[:, :, 0, :])

            # y-inverse: V = [Vr; Vi] = L1.T @ CEy + L2.T @ SEy
            v_ps = ppool.tile([W, SPC, D], f32)
            for s in range(SPC):
                nc.tensor.matmul(v_ps[:, s, :], t_l1[:, s, :, :],
                                 t_bl[0:16, G_CEY:G_CEY + D], start=True, stop=False)
                nc.tensor.matmul(v_ps[:, s, :], t_l2[:, s, :, :],
                                 t_bl[0:16, G_SEY:G_SEY + D], start=False, stop=True)
            t_v = wpool.tile([W + 1, SPC, D], f32)
            # V row 66 = c_total; paired with the ones row 66 of the extended
            # x-inverse matrix this adds the scalar bias inside the matmul.
            # Engine partition bases must be 32-aligned, so copy rows 64:67
            # from the blob first and let the V copy overwrite rows 64:65.
            nc.vector.tensor_copy(t_v[64:W + 1, :, :],
                                  t_bl[64:W + 1, G_CROW:G_CROW + SPC * D])
            nc.vector.tensor_copy(t_v[0:W, :, :], v_ps[:])

            # x-inverse (incl. bias row), then out = s_base * z + O fused
            o_ps = ppool.tile([D, SPC, D], f32)
            for s in range(SPC):
                nc.tensor.matmul(o_ps[:, s, :], t_bl[0:W + 1, G_ICIS:G_ICIS + D],
                                 t_v[:, s, :], start=True, stop=True)
            t_out = wpool.tile([D, SPC, D], f32)
            nc.vector.scalar_tensor_tensor(
                t_out[:], t_z[:], t_bl[:, G_SB:G_SB + 1], o_ps[:],
                op0=ALU.mult, op1=ALU.add)

            nc.sync.dma_start(outc.rearrange("s x y -> x s y"), t_out[:])

    nc.finalize()
    return nc


def _run_device(z, consts):
    global LAST_RESULT
    import os
    from concourse.bass_utils import run_bass_kernel_spmd

    raw = os.environ.get("FNDE_IMPL", "raw") == "raw"
    key = "raw" if raw else "tile"
    if key not in _NC_CACHE:
        _NC_CACHE[key] = _build_nc_raw() if raw else _build_nc()
    nc = _NC_CACHE[key]
    in_maps = _shard_inputs_raw(z, consts) if raw else _shard_inputs(z, consts)
    res = run_bass_kernel_spmd(nc, in_maps, core_ids=list(range(NCORES)))
    LAST_RESULT = res
    out = np.empty((B, 1, D, D), np.float32)
    for i in range(NCORES):
        out[i * SPC:(i + 1) * SPC, 0] = res.results[i]["outc"]
    return out


def kernel(z, lift_w, lift_b, spec_w1, spec_w2, fl_bias, p1_w, p1_b, p2_w, p2_b,
           samp_ts):
    inputs = dict(z=z, lift_w=lift_w, lift_b=lift_b, spec_w1=spec_w1,
                  spec_w2=spec_w2, fl_bias=fl_bias, p1_w=p1_w, p1_b=p1_b,
                  p2_w=p2_w, p2_b=p2_b, samp_ts=samp_ts)
    consts = _fold_weights(inputs)
    z = np.asarray(z, np.float32)
    return _run_device(z, consts)


def kernel_numpy(z, **kw):
    """Pure-numpy path running the same folded math (validation only)."""
    inputs = dict(z=z, **kw)
    consts = _fold_weights(inputs)
    z = np.asarray(z, np.float32)
    out = np.empty((B, 1, D, D), np.float32)
    for i in range(NCORES):
        out[i * SPC:(i + 1) * SPC, 0] = _device_sim(z[i * SPC:(i + 1) * SPC, 0], consts)
    return out


# revision 22
# speedup vs baseline: 1.5522x; 1.0053x over previous
"""Trainium2 Bass kernel for nn_FNDE (FNO neural-ODE).

Mathematical collapse (validated to ~5e-7 rel err vs the f32 jax reference):
Each Fourier layer's RK4 integrates dz/dt = f(z) where f (the FNO spectral
conv) is affine, and *linear per rfft2 mode*: retained modes evolve through a
CxC complex matrix, non-retained modes are untouched, the bias only feeds mode
(0,0).  Lift (1->C) and projection (C->64->1) are pointwise linear.  Hence the
whole network is a per-mode complex scalar acting on rfft2(z):

    out[b] = s_base * z[b] + irfft2(S' * rfft2(z[b]))  + c_total

with S' supported on 33 x-rows (kx in [0..16] u [112..127]) x 16 y-cols.
The ky=0 column needs care: irfft2's implicit Hermitian projection couples the
two retained row blocks (top via W1, bottom via conj(W2)) each evaluation.

The device kernel evaluates the restricted DFT chain as TensorEngine matmuls
per sample (data-parallel: 2 samples per core across 8 cores):
    PT  = z.T @ FxC                  (x-DFT, re/im fused, output transposed)
    QQT = CyS.T @ PT                 (y-DFT cos/sin blocks)
    Q   = combine(QQT)               (complex assembly, transposed layout)
    W'  = S' (.) Q                   (elementwise spectral multiplier)
    V   = [W'r;W'i] via L1.T@CEy + L2.T@SEy   (y-inverse)
    O   = ICIS.T @ V + s_base * z    (x-inverse + identity path, PSUM accum)
    out = O + c_total
"""

import numpy as np

B, C, D, M, L = 16, 64, 128, 16, 3
NCORES = 8
SPC = B // NCORES  # samples per core
KXS = np.concatenate([np.arange(17), np.arange(112, 128)])  # 33 retained rows
NR = len(KXS)  # 33


# ----------------------------------------------------------------------------
# host-side weight folding (numpy, float64)
# ----------------------------------------------------------------------------

def _rk4(f, x, ts):
    for i in range(len(ts) - 1):
        h = ts[i + 1] - ts[i]
        k1 = f(x)
        k2 = f(x + 0.5 * h * k1)
        k3 = f(x + 0.5 * h * k2)
        k4 = f(x + h * k3)
        x = x + (h / 6.0) * (k1 + 2 * k2 + 2 * k3 + k4)
    return x


def _fold_weights(inputs):
    lift_w = np.asarray(inputs["lift_w"], np.float64)[:, 0]      # [C]
    lift_b = np.asarray(inputs["lift_b"], np.float64)            # [C]
    w1 = np.asarray(inputs["spec_w1"], np.float64)               # [L,C,C,M,M,2]
    w2 = np.asarray(inputs["spec_w2"], np.float64)
    fl_bias = np.asarray(inputs["fl_bias"], np.float64)          # [L,C]
    p1_w = np.asarray(inputs["p1_w"], np.float64)
    p1_b = np.asarray(inputs["p1_b"], np.float64)
    p2_w = np.asarray(inputs["p2_w"], np.float64)
    p2_b = np.asarray(inputs["p2_b"], np.float64)
    ts = np.asarray(inputs["samp_ts"], np.float64)

    q = (p2_w @ p1_w)[0]                                         # [C]
    c_proj = float((p2_w @ p1_b + p2_b)[0])
    s_base = float(q @ lift_w)

    w1c = w1[..., 0] + 1j * w1[..., 1]                           # [L,C(i),C(o),M,M]
    w2c = w2[..., 0] + 1j * w2[..., 1]
    # einsum "bixy,ioxy->boxy": out_o = sum_i v_i W[i,o]  =>  generator = W^T
    G1 = np.transpose(w1c, (0, 4, 3, 2, 1))                      # [L,ky,kx,o,i]
    G2 = np.transpose(w2c, (0, 4, 3, 2, 1))                      # [L,ky,r,o,i] rows 112+r

    qc = q.astype(np.complex128)

    def chain(G_stack):
        # propagate lift_w through L layers of 4-step RK4 (linear, no bias),
        # then project with q -> per-mode scalar
        x = np.broadcast_to(lift_w, G_stack.shape[1:-2] + (C,)).astype(np.complex128)
        for layer in range(L):
            Gl = G_stack[layer]
            x = _rk4(lambda v: np.einsum("...ij,...j->...i", Gl, v), x, ts)
        return np.einsum("c,...c->...", qc, x)

    s_top = chain(G1[:, 1:])                                     # [15(ky=1..15),16(kx)]
    s_bot = chain(G2[:, 1:])                                     # [15,16(r)]

    # ky = 0 column: Hermitian projection couples the blocks. Independent
    # coords m in [0,16]; generators per layer:
    G0 = np.zeros((L, 17, C, C), np.complex128)
    for layer in range(L):
        G0[layer, 0] = np.real(G1[layer, 0, 0])
        for m in range(1, 16):
            G0[layer, m] = 0.5 * (G1[layer, 0, m] + np.conj(G2[layer, 0, 16 - m]))
        G0[layer, 16] = 0.5 * np.conj(G2[layer, 0, 0])
    s_col0 = chain(G0)                                           # [17]

    # affine offset at mode (0,0): propagate lift_b*D^2 with layer biases
    y = (lift_b * D * D).astype(np.complex128)
    for layer in range(L):
        Gl = G0[layer, 0]
        bl = (fl_bias[layer] * D * D).astype(np.complex128)
        y = _rk4(lambda v: Gl @ v + bl, y, ts)
    t_const = float(np.real(qc @ y))
    c_total = c_proj + t_const / (D * D)

    # assemble restricted multiplier S' = S - s_base on [33 rows, 16 cols]
    Sp = np.full((NR, 16), s_base, np.complex128)
    Sp[0:16, 1:16] = s_top.T                                     # [kx, ky]
    Sp[17:33, 1:16] = s_bot.T
    Sp[0:17, 0] = s_col0
    for r in range(16):                                          # stored bottom of ky=0
        Sp[17 + r, 0] = np.conj(s_col0[16 - r])
    Sp = Sp - s_base

    # ------------------------------------------------------------------
    # constant matrices for the device matmul chain (float32)
    # ------------------------------------------------------------------
    xg = np.arange(D, dtype=np.float64)
    th_x = 2.0 * np.pi * np.outer(xg, KXS) / D                   # [x, r]
    fxc = np.concatenate([np.cos(th_x), -np.sin(th_x)], axis=1)  # [128, 66]

    ky = np.arange(16, dtype=np.float64)
    th_y = 2.0 * np.pi * np.outer(xg, ky) / D                    # [y, ky]
    cys = np.concatenate([np.cos(th_y), np.sin(th_y)], axis=1)   # [128, 32]

    cc = np.where(ky == 0, 1.0, 2.0) / D                         # [16]
    cey = cc[:, None] * np.cos(th_y.T)                           # [16, 128]
    sey = cc[:, None] * np.sin(th_y.T)

    icis = np.concatenate([np.cos(th_x.T), -np.sin(th_x.T)], axis=0) / D  # [66,128]

    srt = Sp.real.T.astype(np.float32)                           # [16, 33]
    sit = Sp.imag.T.astype(np.float32)

    return dict(
        fxc=fxc.astype(np.float32),
        cys=cys.astype(np.float32),
        cey=cey.astype(np.float32),
        sey=sey.astype(np.float32),
        icis=icis.astype(np.float32),
        srt4=np.tile(srt, (1, 2 * SPC)).astype(np.float32),      # [16, 132]
        sit4=np.tile(sit, (1, 2 * SPC)).astype(np.float32),
        s_base=np.float32(s_base),
        c_total=np.float32(c_total),
    )


# single constant blob [128, GW] — full 128 partitions so the HWDGE splits
# the transfer across all 16 DMA engines (odd partition counts degrade to a
# single-queue chunked transfer)
G_FXC, G_CYC, G_CYS, G_SB = 0, 66, 82, 98
G_ICIS, G_CEY, G_SEY, G_SRT, G_SIT, G_CROW = 99, 227, 355, 483, 615, 747
GW = 1003


def _pack_blobs(k):
    bl = np.zeros((D, GW), np.float32)
    bl[:, G_FXC:G_FXC + 2 * NR] = k["fxc"]
    bl[:, G_CYC:G_CYC + 32] = k["cys"]
    bl[:, G_SB] = k["s_base"]
    bl[0:66, G_ICIS:G_ICIS + D] = k["icis"]
    bl[66, G_ICIS:G_ICIS + D] = 1.0        # ones row of the extended x-inverse
    bl[0:16, G_CEY:G_CEY + D] = k["cey"]
    bl[0:16, G_SEY:G_SEY + D] = k["sey"]
    bl[0:16, G_SRT:G_SRT + 2 * NR * SPC] = k["srt4"]
    bl[0:16, G_SIT:G_SIT + 2 * NR * SPC] = k["sit4"]
    bl[66, G_CROW:G_CROW + SPC * D] = k["c_total"]  # V row 66 -> + c_total
    return bl


# ----------------------------------------------------------------------------
# numpy simulation of the exact device chain (for validation / fallback)
# ----------------------------------------------------------------------------

def _device_sim(z2, k):
    """z2: [SPC,128,128] f32; k: folded consts. Mirrors the Bass kernel."""
    out = np.empty_like(z2)
    for s in range(SPC):
        z = z2[s]
        pt = z.T @ k["fxc"]                                      # [y, 66]
        qc = k["cys"][:, 0:16].T @ pt                            # [16, 66]
        qs = k["cys"][:, 16:32].T @ pt
        qr = qc[:, 0:33] + qs[:, 33:66]                          # [ky, r]
        qi = qc[:, 33:66] - qs[:, 0:33]
        srt = k["srt4"][:, 0:NR]
        sit = k["sit4"][:, 0:NR]
        wr = srt * qr - sit * qi
        wi = srt * qi + sit * qr
        l1 = np.concatenate([wr, wi], axis=1)                    # [16, 66]
        l2 = np.concatenate([-wi, wr], axis=1)
        v = l1.T @ k["cey"] + l2.T @ k["sey"]                    # [66, 128]
        o = k["icis"].T @ v + k["s_base"] * z                    # [128, 128]
        out[s] = o + k["c_total"]
    return out


def _shard_inputs(z, consts):
    bl = _pack_blobs(consts)
    return [
        {"zc": np.ascontiguousarray(z[i * SPC:(i + 1) * SPC, 0]), "bl": bl}
        for i in range(NCORES)
    ]




# ----------------------------------------------------------------------------
# raw (non-Tile) Bass kernel: hand-scheduled semaphores, no Tile tail barrier
# ----------------------------------------------------------------------------

# blob 1 (fp16) [128, B1W]: DFT matrices.  blob 2 (fp32) [128, B2W]: spectral
# multipliers (kept fp32 for the DVE stages), bias row and s_base column.
B1W = 482
R_FXC, R_CYC, R_CYS, R_ICIS, R_CEY, R_SEY = 0, 66, 82, 98, 226, 354
B2W = 68
R_SRT, R_SIT, R_C3, R_SB = 0, 33, 66, 67


def _pack_blobs_raw(k):
    b1 = np.zeros((D, B1W), np.float16)
    b1[:, R_FXC:R_FXC + 2 * NR] = k["fxc"].astype(np.float16)
    b1[:, R_CYC:R_CYC + 32] = k["cys"].astype(np.float16)
    b1[0:66, R_ICIS:R_ICIS + D] = k["icis"].astype(np.float16)
    b1[66, R_ICIS:R_ICIS + D] = 1.0
    b1[0:16, R_CEY:R_CEY + D] = k["cey"].astype(np.float16)
    b1[0:16, R_SEY:R_SEY + D] = k["sey"].astype(np.float16)
    b2 = np.zeros((D, B2W), np.float32)
    b2[0:16, R_SRT:R_SRT + NR] = k["srt4"][:, 0:NR]
    b2[0:16, R_SIT:R_SIT + NR] = k["sit4"][:, 0:NR]
    b2[64:67, R_C3] = k["c_total"]
    b2[:, R_SB] = k["s_base"]
    return b1, b2


def _build_nc_raw():
    import concourse.bacc as bacc
    import concourse.mybir as mybir

    f32 = mybir.dt.float32
    f16 = mybir.dt.float16
    ALU = mybir.AluOpType
    nc = bacc.Bacc("TRN2", target_bir_lowering=False, debug=False,
                   num_devices=NCORES)
    zc = nc.dram_tensor("zc", [SPC, D, D], f32, kind="ExternalInput")
    b1 = nc.dram_tensor("b1", [D, B1W], f16, kind="ExternalInput")
    b2 = nc.dram_tensor("b2", [D, B2W], f32, kind="ExternalInput")
    outc = nc.dram_tensor("outc", [SPC, D, D], f32, kind="ExternalOutput")

    W = 2 * NR  # 66

    t_b1 = nc.alloc_sbuf_tensor("t_b1", [D, B1W], f16).ap()
    t_b2 = nc.alloc_sbuf_tensor("t_b2", [D, B2W], f32).ap()
    t_z = nc.alloc_sbuf_tensor("t_z", [D, SPC, D], f32).ap()
    t_z16 = nc.alloc_sbuf_tensor("t_z16", [D, SPC, D], f16).ap()
    t_ptb = nc.alloc_sbuf_tensor("t_ptb", [D, SPC, W], f16).ap()
    t_qqs = nc.alloc_sbuf_tensor("t_qqs", [16, SPC, W], f32).ap()
    t_qt = nc.alloc_sbuf_tensor("t_qt", [16, SPC, 2, NR], f32).ap()
    t_m1 = nc.alloc_sbuf_tensor("t_m1", [16, SPC, 2, NR], f32).ap()
    t_m2 = nc.alloc_sbuf_tensor("t_m2", [16, SPC, 2, NR], f32).ap()
    t_l1 = nc.alloc_sbuf_tensor("t_l1", [16, SPC, 2, NR], f16).ap()
    t_l2 = nc.alloc_sbuf_tensor("t_l2", [16, SPC, 2, NR], f16).ap()
    t_v = nc.alloc_sbuf_tensor("t_v", [W + 1, SPC, D], f16).ap()
    t_out = nc.alloc_sbuf_tensor("t_out", [D, SPC, D], f32).ap()
    pt_ps = nc.alloc_psum_tensor("pt_ps", [D, SPC, W], f32).ap()
    qq_ps = nc.alloc_psum_tensor("qq_ps", [16, 2, SPC, W], f32).ap()
    v_ps = nc.alloc_psum_tensor("v_ps", [W, SPC, D], f32).ap()
    o_ps = nc.alloc_psum_tensor("o_ps", [D, SPC, D], f32).ap()

    srt = t_b2[0:16, R_SRT:R_SRT + NR].rearrange(
        "k (a b r) -> k a b r", a=1, b=1).broadcast_to([16, SPC, 2, NR])
    sit = t_b2[0:16, R_SIT:R_SIT + NR].rearrange(
        "k (a b r) -> k a b r", a=1, b=1).broadcast_to([16, SPC, 2, NR])

    with (
        nc.semaphore("s_z") as s_z,
        nc.semaphore("s_b1") as s_b1,
        nc.semaphore("s_b2") as s_b2,
        nc.semaphore("s_pe") as s_pe,
        nc.semaphore("s_dve") as s_dve,
        nc.semaphore("s_fin") as s_fin,
        nc.semaphore("s_od") as s_od,
        nc.Block() as block,
    ):
        @block.sync
        def _(sync):
            sync.dma_start(t_z, zc.rearrange("s x y -> x s y")).then_inc(s_z, 16)
            sync.wait_ge(s_fin, 1)
            sync.dma_start(outc.rearrange("s x y -> x s y"), t_out).then_inc(
                s_od, 16)
            sync.wait_ge(s_od, 16)

        @block.scalar
        def _(scalar):
            scalar.dma_start(t_b1, b1[:]).then_inc(s_b1, 16)
            scalar.dma_start(t_b2, b2[:]).then_inc(s_b2, 16)

        @block.tensor
        def _(tensor):
            tensor.wait_ge(s_b1, 16)
            tensor.wait_ge(s_dve, 1)         # z16 cast
            for s in range(SPC):
                tensor.matmul(pt_ps[:, s, :], t_z16[:, s, :],
                              t_b1[:, R_FXC:R_FXC + W],
                              start=True, stop=True).then_inc(s_pe, 1)
            tensor.wait_ge(s_dve, 2)         # ptb ready
            tensor.matmul(qq_ps[:, 0, :, :], t_b1[:, R_CYC:R_CYC + 16],
                          t_ptb, start=True, stop=True).then_inc(s_pe, 1)
            tensor.matmul(qq_ps[:, 1, :, :], t_b1[:, R_CYS:R_CYS + 16],
                          t_ptb, start=True, stop=True).then_inc(s_pe, 1)
            tensor.wait_ge(s_dve, 9)         # l1 ready
            tensor.matmul(v_ps[:, 0, :], t_l1[:, 0, :, :],
                          t_b1[0:16, R_CEY:R_CEY + D],
                          start=True, stop=False).then_inc(s_pe, 1)
            tensor.wait_ge(s_dve, 11)        # l2 ready
            tensor.matmul(v_ps[:, 0, :], t_l2[:, 0, :, :],
                          t_b1[0:16, R_SEY:R_SEY + D],
                          start=False, stop=True).then_inc(s_pe, 1)
            tensor.matmul(v_ps[:, 1, :], t_l1[:, 1, :, :],
                          t_b1[0:16, R_CEY:R_CEY + D],
                          start=True, stop=False).then_inc(s_pe, 1)
            tensor.matmul(v_ps[:, 1, :], t_l2[:, 1, :, :],
                          t_b1[0:16, R_SEY:R_SEY + D],
                          start=False, stop=True).then_inc(s_pe, 1)
            tensor.wait_ge(s_dve, 13)        # t_v assembled
            for s in range(SPC):
                tensor.matmul(o_ps[:, s, :], t_b1[0:W + 1, R_ICIS:R_ICIS + D],
                              t_v[:, s, :],
                              start=True, stop=True).then_inc(s_pe, 1)

        @block.vector
        def _(vector):
            # NB: the DVE pipeline overlaps its own instructions, so
            # same-engine RAW/WAW also needs explicit s_dve waits.
            vector.wait_ge(s_z, 16)
            vector.tensor_copy(t_z16, t_z).then_inc(s_dve, 1)          # 1
            vector.wait_ge(s_pe, 2)
            vector.tensor_copy(t_ptb, pt_ps).then_inc(s_dve, 1)        # 2
            vector.wait_ge(s_pe, 4)
            vector.tensor_copy(t_qqs, qq_ps[:, 1, :, :]).then_inc(s_dve, 1)  # 3
            vector.wait_ge(s_dve, 3)
            vector.tensor_add(t_qt[:, :, 0, :], qq_ps[:, 0, :, 0:NR],
                              t_qqs[:, :, NR:W]).then_inc(s_dve, 1)    # 4
            vector.tensor_sub(t_qt[:, :, 1, :], qq_ps[:, 0, :, NR:W],
                              t_qqs[:, :, 0:NR]).then_inc(s_dve, 1)    # 5
            vector.wait_ge(s_b2, 16)
            vector.wait_ge(s_dve, 5)
            vector.tensor_mul(t_m1, srt, t_qt).then_inc(s_dve, 1)      # 6
            vector.tensor_mul(t_m2, sit, t_qt).then_inc(s_dve, 1)      # 7
            vector.wait_ge(s_dve, 7)
            vector.tensor_sub(t_l1[:, :, 0, :], t_m1[:, :, 0, :],
                              t_m2[:, :, 1, :]).then_inc(s_dve, 1)     # 8
            vector.tensor_add(t_l1[:, :, 1, :], t_m1[:, :, 1, :],
                              t_m2[:, :, 0, :]).then_inc(s_dve, 1)     # 9
            vector.wait_ge(s_dve, 9)
            vector.tensor_scalar_mul(t_l2[:, :, 0, :], t_l1[:, :, 1, :],
                                     -1.0).then_inc(s_dve, 1)          # 10
            vector.tensor_copy(t_l2[:, :, 1, :],
                               t_l1[:, :, 0, :]).then_inc(s_dve, 1)    # 11
            vector.tensor_scalar(t_v[64:W + 1, :, :], t_z[64:67, :, :],
                                 0.0, t_b2[64:67, R_C3:R_C3 + 1],
                                 op0=ALU.mult, op1=ALU.add
                                 ).then_inc(s_dve, 1)                  # 12
            vector.wait_ge(s_pe, 8)
            vector.wait_ge(s_dve, 12)
            vector.tensor_copy(t_v[0:W, :, :], v_ps).then_inc(s_dve, 1)  # 13
            vector.wait_ge(s_pe, 10)
            vector.scalar_tensor_tensor(t_out, t_z, t_b2[:, R_SB:R_SB + 1],
                                        o_ps, op0=ALU.mult,
                                        op1=ALU.add).then_inc(s_fin, 1)

    nc.finalize()
    return nc


def _shard_inputs_raw(z, consts):
    b1, b2 = _pack_blobs_raw(consts)
    return [
        {"zc": np.ascontiguousarray(z[i * SPC:(i + 1) * SPC, 0]),
         "b1": b1, "b2": b2}
        for i in range(NCORES)
    ]


# ----------------------------------------------------------------------------
# Bass kernel
# ----------------------------------------------------------------------------

_NC_CACHE = {}
LAST_RESULT = None  # BassKernelResults of the most recent device run


def _build_nc():
    import concourse.bacc as bacc
    import concourse.mybir as mybir
    import concourse.tile as tile

    f32 = mybir.dt.float32
    ALU = mybir.AluOpType
    nc = bacc.Bacc("TRN2", target_bir_lowering=False, debug=False,
                   num_devices=NCORES)
    zc = nc.dram_tensor("zc", [SPC, D, D], f32, kind="ExternalInput")
    bl = nc.dram_tensor("bl", [D, GW], f32, kind="ExternalInput")
    outc = nc.dram_tensor("outc", [SPC, D, D], f32, kind="ExternalOutput")

    W = 2 * NR  # 66

    with tile.TileContext(nc) as tc:
        with (
            tc.tile_pool(name="const", bufs=1) as cpool,
            tc.tile_pool(name="work", bufs=2) as wpool,
            tc.tile_pool(name="psum", bufs=1, space="PSUM") as ppool,
        ):
            # constants arrive as one packed blob, issued from the otherwise-
            # idle scalar sequencer so the z DMA on sync runs in parallel
            t_bl = cpool.tile([D, GW], f32)
            nc.scalar.dma_start(t_bl[:], bl[:])

            t_z = wpool.tile([D, SPC, D], f32)
            nc.sync.dma_start(t_z[:], zc.rearrange("s x y -> x s y"))

            # x-forward DFT (output already transposed): PT_s = z_s.T @ FxC
            pt_ps = ppool.tile([D, SPC, W], f32)
            for s in range(SPC):
                nc.tensor.matmul(pt_ps[:, s, :], t_z[:, s, :],
                                 t_bl[:, G_FXC:G_FXC + W], start=True, stop=True)
            t_ptb = wpool.tile([D, SPC, W], f32)
            nc.vector.tensor_copy(t_ptb[:], pt_ps[:])

            # y-forward DFT, cos and sin blocks both on partitions 0:16
            # (two-input DVE ops require equal base partitions)
            qq_ps = ppool.tile([16, 2, SPC, W], f32)
            nc.tensor.matmul(qq_ps[:, 0, :, :], t_bl[:, G_CYC:G_CYC + 16],
                             t_ptb[:], start=True, stop=True)
            nc.tensor.matmul(qq_ps[:, 1, :, :], t_bl[:, G_CYS:G_CYS + 16],
                             t_ptb[:], start=True, stop=True)
            t_qq = wpool.tile([16, 2, SPC, W], f32)
            nc.vector.tensor_copy(t_qq[:], qq_ps[:])

            # complex assembly (transposed layout [ky, r]), batched over samples
            t_qt = wpool.tile([16, SPC, 2, NR], f32)
            nc.vector.tensor_add(t_qt[:, :, 0, :],
                                 t_qq[:, 0, :, 0:NR], t_qq[:, 1, :, NR:W])
            nc.vector.tensor_sub(t_qt[:, :, 1, :],
                                 t_qq[:, 0, :, NR:W], t_qq[:, 1, :, 0:NR])

            # spectral multiplier: W'r = Sr*Qr - Si*Qi ; W'i = Sr*Qi + Si*Qr
            srt = t_bl[0:16, G_SRT:G_SRT + W * SPC].rearrange(
                "k (s b r) -> k s b r", s=SPC, b=2)
            sit = t_bl[0:16, G_SIT:G_SIT + W * SPC].rearrange(
                "k (s b r) -> k s b r", s=SPC, b=2)
            t_m1 = wpool.tile([16, SPC, 2, NR], f32)
            t_m2 = wpool.tile([16, SPC, 2, NR], f32)
            nc.vector.tensor_mul(t_m1[:], srt, t_qt[:])
            nc.vector.tensor_mul(t_m2[:], sit, t_qt[:])
            t_l1 = wpool.tile([16, SPC, 2, NR], f32)
            t_l2 = wpool.tile([16, SPC, 2, NR], f32)
            nc.vector.tensor_sub(t_l1[:, :, 0, :], t_m1[:, :, 0, :], t_m2[:, :, 1, :])
            nc.vector.tensor_add(t_l1[:, :, 1, :], t_m1[:, :, 1, :], t_m2[:, :, 0, :])
            nc.vector.tensor_scalar_mul(t_l2[:, :, 0, :], t_l1[:, :, 1, :], -1.0)
            nc.vector.tensor_copy(t_l2[:, :, 1, :], t_l1[:, :, 0, :])

            # y-inverse: V = [Vr; Vi] = L1.T @ CEy + L2.T @ SEy
            v_ps = ppool.tile([W, SPC, D], f32)
            for s in range(SPC):
                nc.tensor.matmul(v_ps[:, s, :], t_l1[:, s, :, :],
                                 t_bl[0:16, G_CEY:G_CEY + D], start=True, stop=False)
                nc.tensor.matmul(v_ps[:, s, :], t_l2[:, s, :, :],
                                 t_bl[0:16, G_SEY:G_SEY + D], start=False, stop=True)
            t_v = wpool.tile([W + 1, SPC, D], f32)
            # V row 66 = c_total; paired with the ones row 66 of the extended
            # x-inverse matrix this adds the scalar bias inside the matmul.
            # Engine partition bases must be 32-aligned, so copy rows 64:67
            # from the blob first and let the V copy overwrite rows 64:65.
            nc.vector.tensor_copy(t_v[64:W + 1, :, :],
                                  t_bl[64:W + 1, G_CROW:G_CROW + SPC * D])
            nc.vector.tensor_copy(t_v[0:W, :, :], v_ps[:])

            # x-inverse (incl. bias row), then out = s_base * z + O fused
            o_ps = ppool.tile([D, SPC, D], f32)
            for s in range(SPC):
                nc.tensor.matmul(o_ps[:, s, :], t_bl[0:W + 1, G_ICIS:G_ICIS + D],
                                 t_v[:, s, :], start=True, stop=True)
            t_out = wpool.tile([D, SPC, D], f32)
            nc.vector.scalar_tensor_tensor(
                t_out[:], t_z[:], t_bl[:, G_SB:G_SB + 1], o_ps[:],
                op0=ALU.mult, op1=ALU.add)

            nc.sync.dma_start(outc.rearrange("s x y -> x s y"), t_out[:])

    nc.finalize()
    return nc


def _run_device(z, consts):
    global LAST_RESULT
    import os
    from concourse.bass_utils import run_bass_kernel_spmd

    raw = os.environ.get("FNDE_IMPL", "raw") == "raw"
    key = "raw" if raw else "tile"
    if key not in _NC_CACHE:
        _NC_CACHE[key] = _build_nc_raw() if raw else _build_nc()
    nc = _NC_CACHE[key]
    in_maps = _shard_inputs_raw(z, consts) if raw else _shard_inputs(z, consts)
    res = run_bass_kernel_spmd(nc, in_maps, core_ids=list(range(NCORES)))
    LAST_RESULT = res
    out = np.empty((B, 1, D, D), np.float32)
    for i in range(NCORES):
        out[i * SPC:(i + 1) * SPC, 0] = res.results[i]["outc"]
    return out


def kernel(z, lift_w, lift_b, spec_w1, spec_w2, fl_bias, p1_w, p1_b, p2_w, p2_b,
           samp_ts):
    inputs = dict(z=z, lift_w=lift_w, lift_b=lift_b, spec_w1=spec_w1,
                  spec_w2=spec_w2, fl_bias=fl_bias, p1_w=p1_w, p1_b=p1_b,
                  p2_w=p2_w, p2_b=p2_b, samp_ts=samp_ts)
    consts = _fold_weights(inputs)
    z = np.asarray(z, np.float32)
    return _run_device(z, consts)


def kernel_numpy(z, **kw):
    """Pure-numpy path running the same folded math (validation only)."""
    inputs = dict(z=z, **kw)
    consts = _fold_weights(inputs)
    z = np.asarray(z, np.float32)
    out = np.empty((B, 1, D, D), np.float32)
    for i in range(NCORES):
        out[i * SPC:(i + 1) * SPC, 0] = _device_sim(z[i * SPC:(i + 1) * SPC, 0], consts)
    return out


# revision 24
# speedup vs baseline: 1.6038x; 1.0332x over previous
"""Trainium2 Bass kernel for nn_FNDE (FNO neural-ODE).

Mathematical collapse (validated to ~5e-7 rel err vs the f32 jax reference):
Each Fourier layer's RK4 integrates dz/dt = f(z) where f (the FNO spectral
conv) is affine, and *linear per rfft2 mode*: retained modes evolve through a
CxC complex matrix, non-retained modes are untouched, the bias only feeds mode
(0,0).  Lift (1->C) and projection (C->64->1) are pointwise linear.  Hence the
whole network is a per-mode complex scalar acting on rfft2(z):

    out[b] = s_base * z[b] + irfft2(S' * rfft2(z[b]))  + c_total

with S' supported on 33 x-rows (kx in [0..16] u [112..127]) x 16 y-cols.
The ky=0 column needs care: irfft2's implicit Hermitian projection couples the
two retained row blocks (top via W1, bottom via conj(W2)) each evaluation.

The device kernel evaluates the restricted DFT chain as TensorEngine matmuls
per sample (data-parallel: 2 samples per core across 8 cores):
    PT  = z.T @ FxC                  (x-DFT, re/im fused, output transposed)
    QQT = CyS.T @ PT                 (y-DFT cos/sin blocks)
    Q   = combine(QQT)               (complex assembly, transposed layout)
    W'  = S' (.) Q                   (elementwise spectral multiplier)
    V   = [W'r;W'i] via L1.T@CEy + L2.T@SEy   (y-inverse)
    O   = ICIS.T @ V + s_base * z    (x-inverse + identity path, PSUM accum)
    out = O + c_total
"""

import numpy as np

B, C, D, M, L = 16, 64, 128, 16, 3
NCORES = 8
SPC = B // NCORES  # samples per core
KXS = np.concatenate([np.arange(17), np.arange(112, 128)])  # 33 retained rows
NR = len(KXS)  # 33


# ----------------------------------------------------------------------------
# host-side weight folding (numpy, float64)
# ----------------------------------------------------------------------------

def _rk4(f, x, ts):
    for i in range(len(ts) - 1):
        h = ts[i + 1] - ts[i]
        k1 = f(x)
        k2 = f(x + 0.5 * h * k1)
        k3 = f(x + 0.5 * h * k2)
        k4 = f(x + h * k3)
        x = x + (h / 6.0) * (k1 + 2 * k2 + 2 * k3 + k4)
    return x


def _fold_weights(inputs):
    lift_w = np.asarray(inputs["lift_w"], np.float64)[:, 0]      # [C]
    lift_b = np.asarray(inputs["lift_b"], np.float64)            # [C]
    w1 = np.asarray(inputs["spec_w1"], np.float64)               # [L,C,C,M,M,2]
    w2 = np.asarray(inputs["spec_w2"], np.float64)
    fl_bias = np.asarray(inputs["fl_bias"], np.float64)          # [L,C]
    p1_w = np.asarray(inputs["p1_w"], np.float64)
    p1_b = np.asarray(inputs["p1_b"], np.float64)
    p2_w = np.asarray(inputs["p2_w"], np.float64)
    p2_b = np.asarray(inputs["p2_b"], np.float64)
    ts = np.asarray(inputs["samp_ts"], np.float64)

    q = (p2_w @ p1_w)[0]                                         # [C]
    c_proj = float((p2_w @ p1_b + p2_b)[0])
    s_base = float(q @ lift_w)

    w1c = w1[..., 0] + 1j * w1[..., 1]                           # [L,C(i),C(o),M,M]
    w2c = w2[..., 0] + 1j * w2[..., 1]
    # einsum "bixy,ioxy->boxy": out_o = sum_i v_i W[i,o]  =>  generator = W^T
    G1 = np.transpose(w1c, (0, 4, 3, 2, 1))                      # [L,ky,kx,o,i]
    G2 = np.transpose(w2c, (0, 4, 3, 2, 1))                      # [L,ky,r,o,i] rows 112+r

    qc = q.astype(np.complex128)

    def chain(G_stack):
        # propagate lift_w through L layers of 4-step RK4 (linear, no bias),
        # then project with q -> per-mode scalar
        x = np.broadcast_to(lift_w, G_stack.shape[1:-2] + (C,)).astype(np.complex128)
        for layer in range(L):
            Gl = G_stack[layer]
            x = _rk4(lambda v: np.einsum("...ij,...j->...i", Gl, v), x, ts)
        return np.einsum("c,...c->...", qc, x)

    s_top = chain(G1[:, 1:])                                     # [15(ky=1..15),16(kx)]
    s_bot = chain(G2[:, 1:])                                     # [15,16(r)]

    # ky = 0 column: Hermitian projection couples the blocks. Independent
    # coords m in [0,16]; generators per layer:
    G0 = np.zeros((L, 17, C, C), np.complex128)
    for layer in range(L):
        G0[layer, 0] = np.real(G1[layer, 0, 0])
        for m in range(1, 16):
            G0[layer, m] = 0.5 * (G1[layer, 0, m] + np.conj(G2[layer, 0, 16 - m]))
        G0[layer, 16] = 0.5 * np.conj(G2[layer, 0, 0])
    s_col0 = chain(G0)                                           # [17]

    # affine offset at mode (0,0): propagate lift_b*D^2 with layer biases
    y = (lift_b * D * D).astype(np.complex128)
    for layer in range(L):
        Gl = G0[layer, 0]
        bl = (fl_bias[layer] * D * D).astype(np.complex128)
        y = _rk4(lambda v: Gl @ v + bl, y, ts)
    t_const = float(np.real(qc @ y))
    c_total = c_proj + t_const / (D * D)

    # assemble restricted multiplier S' = S - s_base on [33 rows, 16 cols]
    Sp = np.full((NR, 16), s_base, np.complex128)
    Sp[0:16, 1:16] = s_top.T                                     # [kx, ky]
    Sp[17:33, 1:16] = s_bot.T
    Sp[0:17, 0] = s_col0
    for r in range(16):                                          # stored bottom of ky=0
        Sp[17 + r, 0] = np.conj(s_col0[16 - r])
    Sp = Sp - s_base

    # ------------------------------------------------------------------
    # constant matrices for the device matmul chain (float32)
    # ------------------------------------------------------------------
    xg = np.arange(D, dtype=np.float64)
    th_x = 2.0 * np.pi * np.outer(xg, KXS) / D                   # [x, r]
    fxc = np.concatenate([np.cos(th_x), -np.sin(th_x)], axis=1)  # [128, 66]

    ky = np.arange(16, dtype=np.float64)
    th_y = 2.0 * np.pi * np.outer(xg, ky) / D                    # [y, ky]
    cys = np.concatenate([np.cos(th_y), np.sin(th_y)], axis=1)   # [128, 32]

    cc = np.where(ky == 0, 1.0, 2.0) / D                         # [16]
    cey = cc[:, None] * np.cos(th_y.T)                           # [16, 128]
    sey = cc[:, None] * np.sin(th_y.T)

    icis = np.concatenate([np.cos(th_x.T), -np.sin(th_x.T)], axis=0) / D  # [66,128]

    srt = Sp.real.T.astype(np.float32)                           # [16, 33]
    sit = Sp.imag.T.astype(np.float32)

    return dict(
        fxc=fxc.astype(np.float32),
        cys=cys.astype(np.float32),
        cey=cey.astype(np.float32),
        sey=sey.astype(np.float32),
        icis=icis.astype(np.float32),
        srt4=np.tile(srt, (1, 2 * SPC)).astype(np.float32),      # [16, 132]
        sit4=np.tile(sit, (1, 2 * SPC)).astype(np.float32),
        s_base=np.float32(s_base),
        c_total=np.float32(c_total),
    )


# single constant blob [128, GW] — full 128 partitions so the HWDGE splits
# the transfer across all 16 DMA engines (odd partition counts degrade to a
# single-queue chunked transfer)
G_FXC, G_CYC, G_CYS, G_SB = 0, 66, 82, 98
G_ICIS, G_CEY, G_SEY, G_SRT, G_SIT, G_CROW = 99, 227, 355, 483, 615, 747
GW = 1003


def _pack_blobs(k):
    bl = np.zeros((D, GW), np.float32)
    bl[:, G_FXC:G_FXC + 2 * NR] = k["fxc"]
    bl[:, G_CYC:G_CYC + 32] = k["cys"]
    bl[:, G_SB] = k["s_base"]
    bl[0:66, G_ICIS:G_ICIS + D] = k["icis"]
    bl[66, G_ICIS:G_ICIS + D] = 1.0        # ones row of the extended x-inverse
    bl[0:16, G_CEY:G_CEY + D] = k["cey"]
    bl[0:16, G_SEY:G_SEY + D] = k["sey"]
    bl[0:16, G_SRT:G_SRT + 2 * NR * SPC] = k["srt4"]
    bl[0:16, G_SIT:G_SIT + 2 * NR * SPC] = k["sit4"]
    bl[66, G_CROW:G_CROW + SPC * D] = k["c_total"]  # V row 66 -> + c_total
    return bl


# ----------------------------------------------------------------------------
# numpy simulation of the exact device chain (for validation / fallback)
# ----------------------------------------------------------------------------

def _device_sim(z2, k):
    """z2: [SPC,128,128] f32; k: folded consts. Mirrors the Bass kernel."""
    out = np.empty_like(z2)
    for s in range(SPC):
        z = z2[s]
        pt = z.T @ k["fxc"]                                      # [y, 66]
        qc = k["cys"][:, 0:16].T @ pt                            # [16, 66]
        qs = k["cys"][:, 16:32].T @ pt
        qr = qc[:, 0:33] + qs[:, 33:66]                          # [ky, r]
        qi = qc[:, 33:66] - qs[:, 0:33]
        srt = k["srt4"][:, 0:NR]
        sit = k["sit4"][:, 0:NR]
        wr = srt * qr - sit * qi
        wi = srt * qi + sit * qr
        l1 = np.concatenate([wr, wi], axis=1)                    # [16, 66]
        l2 = np.concatenate([-wi, wr], axis=1)
        v = l1.T @ k["cey"] + l2.T @ k["sey"]                    # [66, 128]
        o = k["icis"].T @ v + k["s_base"] * z                    # [128, 128]
        out[s] = o + k["c_total"]
    return out


def _shard_inputs(z, consts):
    bl = _pack_blobs(consts)
    return [
        {"zc": np.ascontiguousarray(z[i * SPC:(i + 1) * SPC, 0]), "bl": bl}
        for i in range(NCORES)
    ]




# ----------------------------------------------------------------------------
# raw (non-Tile) Bass kernel: hand-scheduled semaphores, no Tile tail barrier
# ----------------------------------------------------------------------------

# blob 1 (fp16) [128, B1W]: DFT matrices.  blob 2 (fp32) [128, B2W]: spectral
# multipliers (kept fp32 for the DVE stages), bias row and s_base column.
B1W = 482
R_FXC, R_CYC, R_CYS, R_ICIS, R_CEY, R_SEY = 0, 66, 82, 98, 226, 354
B2W = 68
R_SRT, R_SIT, R_C3, R_SB = 0, 33, 66, 67


def _pack_blobs_raw(k):
    b1 = np.zeros((D, B1W), np.float16)
    b1[:, R_FXC:R_FXC + 2 * NR] = k["fxc"].astype(np.float16)
    b1[:, R_CYC:R_CYC + 32] = k["cys"].astype(np.float16)
    b1[0:66, R_ICIS:R_ICIS + D] = k["icis"].astype(np.float16)
    b1[66, R_ICIS:R_ICIS + D] = 1.0
    b1[0:16, R_CEY:R_CEY + D] = k["cey"].astype(np.float16)
    b1[0:16, R_SEY:R_SEY + D] = k["sey"].astype(np.float16)
    b2 = np.zeros((D, B2W), np.float32)
    b2[0:16, R_SRT:R_SRT + NR] = k["srt4"][:, 0:NR]
    b2[0:16, R_SIT:R_SIT + NR] = k["sit4"][:, 0:NR]
    b2[64:67, R_C3] = k["c_total"]
    b2[:, R_SB] = k["s_base"]
    return b1, b2


def _build_nc_raw():
    import concourse.bacc as bacc
    import concourse.mybir as mybir

    f32 = mybir.dt.float32
    f16 = mybir.dt.float16
    ALU = mybir.AluOpType
    nc = bacc.Bacc("TRN2", target_bir_lowering=False, debug=False,
                   num_devices=NCORES)
    zc = nc.dram_tensor("zc", [SPC, D, D], f32, kind="ExternalInput")
    b1 = nc.dram_tensor("b1", [D, B1W], f16, kind="ExternalInput")
    b2 = nc.dram_tensor("b2", [D, B2W], f32, kind="ExternalInput")
    outc = nc.dram_tensor("outc", [SPC, D, D], f16, kind="ExternalOutput")

    W = 2 * NR  # 66

    t_b1 = nc.alloc_sbuf_tensor("t_b1", [D, B1W], f16).ap()
    t_b2 = nc.alloc_sbuf_tensor("t_b2", [D, B2W], f32).ap()
    t_z = nc.alloc_sbuf_tensor("t_z", [D, SPC, D], f32).ap()
    t_z16 = nc.alloc_sbuf_tensor("t_z16", [D, SPC, D], f16).ap()
    t_ptb = nc.alloc_sbuf_tensor("t_ptb", [D, SPC, W], f16).ap()
    t_qqs = nc.alloc_sbuf_tensor("t_qqs", [16, SPC, W], f32).ap()
    t_qt = nc.alloc_sbuf_tensor("t_qt", [16, SPC, 2, NR], f32).ap()
    t_m1 = nc.alloc_sbuf_tensor("t_m1", [16, SPC, 2, NR], f32).ap()
    t_m2 = nc.alloc_sbuf_tensor("t_m2", [16, SPC, 2, NR], f32).ap()
    t_l1 = nc.alloc_sbuf_tensor("t_l1", [16, SPC, 2, NR], f16).ap()
    t_l2 = nc.alloc_sbuf_tensor("t_l2", [16, SPC, 2, NR], f16).ap()
    t_v = nc.alloc_sbuf_tensor("t_v", [W + 1, SPC, D], f16).ap()
    t_out = nc.alloc_sbuf_tensor("t_out", [D, SPC, D], f16).ap()
    pt_ps = nc.alloc_psum_tensor("pt_ps", [D, SPC, W], f32).ap()
    qq_ps = nc.alloc_psum_tensor("qq_ps", [16, 2, SPC, W], f32).ap()
    v_ps = nc.alloc_psum_tensor("v_ps", [W, SPC, D], f32).ap()
    o_ps = nc.alloc_psum_tensor("o_ps", [D, SPC, D], f32).ap()

    srt = t_b2[0:16, R_SRT:R_SRT + NR].rearrange(
        "k (a b r) -> k a b r", a=1, b=1).broadcast_to([16, SPC, 2, NR])
    sit = t_b2[0:16, R_SIT:R_SIT + NR].rearrange(
        "k (a b r) -> k a b r", a=1, b=1).broadcast_to([16, SPC, 2, NR])

    with (
        nc.semaphore("s_z") as s_z,
        nc.semaphore("s_b1") as s_b1,
        nc.semaphore("s_b2") as s_b2,
        nc.semaphore("s_pe") as s_pe,
        nc.semaphore("s_dve") as s_dve,
        nc.semaphore("s_fin") as s_fin,
        nc.semaphore("s_od") as s_od,
        nc.Block() as block,
    ):
        @block.sync
        def _(sync):
            sync.dma_start(t_z, zc.rearrange("s x y -> x s y")).then_inc(s_z, 16)
            sync.wait_ge(s_fin, 2)
            sync.dma_start(outc[1],
                           t_out[:, 1, :]).then_inc(s_od, 16)
            sync.wait_ge(s_od, 32)

        @block.scalar
        def _(scalar):
            scalar.dma_start(t_b1, b1[:]).then_inc(s_b1, 16)
            scalar.dma_start(t_b2, b2[:]).then_inc(s_b2, 16)
            scalar.wait_ge(s_fin, 1)
            scalar.dma_start(outc[0],
                             t_out[:, 0, :]).then_inc(s_od, 16)

        @block.tensor
        def _(tensor):
            tensor.wait_ge(s_b1, 16)
            tensor.wait_ge(s_dve, 1)         # z16 cast
            for s in range(SPC):
                tensor.matmul(pt_ps[:, s, :], t_z16[:, s, :],
                              t_b1[:, R_FXC:R_FXC + W],
                              start=True, stop=True).then_inc(s_pe, 1)
            tensor.wait_ge(s_dve, 2)         # ptb ready
            tensor.matmul(qq_ps[:, 0, :, :], t_b1[:, R_CYC:R_CYC + 16],
                          t_ptb, start=True, stop=True).then_inc(s_pe, 1)
            tensor.matmul(qq_ps[:, 1, :, :], t_b1[:, R_CYS:R_CYS + 16],
                          t_ptb, start=True, stop=True).then_inc(s_pe, 1)
            tensor.wait_ge(s_dve, 9)         # l1 ready
            tensor.matmul(v_ps[:, 0, :], t_l1[:, 0, :, :],
                          t_b1[0:16, R_CEY:R_CEY + D],
                          start=True, stop=False).then_inc(s_pe, 1)
            tensor.wait_ge(s_dve, 11)        # l2 ready
            tensor.matmul(v_ps[:, 0, :], t_l2[:, 0, :, :],
                          t_b1[0:16, R_SEY:R_SEY + D],
                          start=False, stop=True).then_inc(s_pe, 1)
            tensor.matmul(v_ps[:, 1, :], t_l1[:, 1, :, :],
                          t_b1[0:16, R_CEY:R_CEY + D],
                          start=True, stop=False).then_inc(s_pe, 1)
            tensor.matmul(v_ps[:, 1, :], t_l2[:, 1, :, :],
                          t_b1[0:16, R_SEY:R_SEY + D],
                          start=False, stop=True).then_inc(s_pe, 1)
            tensor.wait_ge(s_dve, 13)        # t_v assembled
            for s in range(SPC):
                tensor.matmul(o_ps[:, s, :], t_b1[0:W + 1, R_ICIS:R_ICIS + D],
                              t_v[:, s, :],
                              start=True, stop=True).then_inc(s_pe, 1)

        @block.vector
        def _(vector):
            # NB: the DVE pipeline overlaps its own instructions, so
            # same-engine RAW/WAW also needs explicit s_dve waits.
            vector.wait_ge(s_z, 16)
            vector.tensor_copy(t_z16, t_z).then_inc(s_dve, 1)          # 1
            vector.wait_ge(s_pe, 2)
            vector.tensor_copy(t_ptb, pt_ps).then_inc(s_dve, 1)        # 2
            vector.wait_ge(s_pe, 4)
            vector.tensor_copy(t_qqs, qq_ps[:, 1, :, :]).then_inc(s_dve, 1)  # 3
            vector.wait_ge(s_dve, 3)
            vector.tensor_add(t_qt[:, :, 0, :], qq_ps[:, 0, :, 0:NR],
                              t_qqs[:, :, NR:W]).then_inc(s_dve, 1)    # 4
            vector.tensor_sub(t_qt[:, :, 1, :], qq_ps[:, 0, :, NR:W],
                              t_qqs[:, :, 0:NR]).then_inc(s_dve, 1)    # 5
            vector.wait_ge(s_b2, 16)
            vector.wait_ge(s_dve, 5)
            vector.tensor_mul(t_m1, srt, t_qt).then_inc(s_dve, 1)      # 6
            vector.tensor_mul(t_m2, sit, t_qt).then_inc(s_dve, 1)      # 7
            vector.wait_ge(s_dve, 7)
            vector.tensor_sub(t_l1[:, :, 0, :], t_m1[:, :, 0, :],
                              t_m2[:, :, 1, :]).then_inc(s_dve, 1)     # 8
            vector.tensor_add(t_l1[:, :, 1, :], t_m1[:, :, 1, :],
                              t_m2[:, :, 0, :]).then_inc(s_dve, 1)     # 9
            vector.wait_ge(s_dve, 9)
            vector.tensor_scalar_mul(t_l2[:, :, 0, :], t_l1[:, :, 1, :],
                                     -1.0).then_inc(s_dve, 1)          # 10
            vector.tensor_copy(t_l2[:, :, 1, :],
                               t_l1[:, :, 0, :]).then_inc(s_dve, 1)    # 11
            vector.tensor_scalar(t_v[64:W + 1, :, :], t_z[64:67, :, :],
                                 0.0, t_b2[64:67, R_C3:R_C3 + 1],
                                 op0=ALU.mult, op1=ALU.add
                                 ).then_inc(s_dve, 1)                  # 12
            vector.wait_ge(s_pe, 8)
            vector.wait_ge(s_dve, 12)
            vector.tensor_copy(t_v[0:W, :, :], v_ps).then_inc(s_dve, 1)  # 13
            vector.wait_ge(s_pe, 9)
            vector.scalar_tensor_tensor(t_out[:, 0, :], t_z[:, 0, :],
                                        t_b2[:, R_SB:R_SB + 1], o_ps[:, 0, :],
                                        op0=ALU.mult,
                                        op1=ALU.add).then_inc(s_fin, 1)
            vector.wait_ge(s_pe, 10)
            vector.scalar_tensor_tensor(t_out[:, 1, :], t_z[:, 1, :],
                                        t_b2[:, R_SB:R_SB + 1], o_ps[:, 1, :],
                                        op0=ALU.mult,
                                        op1=ALU.add).then_inc(s_fin, 1)

    nc.finalize()
    return nc


def _shard_inputs_raw(z, consts):
    b1, b2 = _pack_blobs_raw(consts)
    return [
        {"zc": np.ascontiguousarray(z[i * SPC:(i + 1) * SPC, 0]),
         "b1": b1, "b2": b2}
        for i in range(NCORES)
    ]


# ----------------------------------------------------------------------------
# Bass kernel
# ----------------------------------------------------------------------------

_NC_CACHE = {}
LAST_RESULT = None  # BassKernelResults of the most recent device run


def _build_nc():
    import concourse.bacc as bacc
    import concourse.mybir as mybir
    import concourse.tile as tile

    f32 = mybir.dt.float32
    ALU = mybir.AluOpType
    nc = bacc.Bacc("TRN2", target_bir_lowering=False, debug=False,
                   num_devices=NCORES)
    zc = nc.dram_tensor("zc", [SPC, D, D], f32, kind="ExternalInput")
    bl = nc.dram_tensor("bl", [D, GW], f32, kind="ExternalInput")
    outc = nc.dram_tensor("outc", [SPC, D, D], f16, kind="ExternalOutput")

    W = 2 * NR  # 66

    with tile.TileContext(nc) as tc:
        with (
            tc.tile_pool(name="const", bufs=1) as cpool,
            tc.tile_pool(name="work", bufs=2) as wpool,
            tc.tile_pool(name="psum", bufs=1, space="PSUM") as ppool,
        ):
            # constants arrive as one packed blob, issued from the otherwise-
            # idle scalar sequencer so the z DMA on sync runs in parallel
            t_bl = cpool.tile([D, GW], f32)
            nc.scalar.dma_start(t_bl[:], bl[:])

            t_z = wpool.tile([D, SPC, D], f32)
            nc.sync.dma_start(t_z[:], zc.rearrange("s x y -> x s y"))

            # x-forward DFT (output already transposed): PT_s = z_s.T @ FxC
            pt_ps = ppool.tile([D, SPC, W], f32)
            for s in range(SPC):
                nc.tensor.matmul(pt_ps[:, s, :], t_z[:, s, :],
                                 t_bl[:, G_FXC:G_FXC + W], start=True, stop=True)
            t_ptb = wpool.tile([D, SPC, W], f32)
            nc.vector.tensor_copy(t_ptb[:], pt_ps[:])

            # y-forward DFT, cos and sin blocks both on partitions 0:16
            # (two-input DVE ops require equal base partitions)
            qq_ps = ppool.tile([16, 2, SPC, W], f32)
            nc.tensor.matmul(qq_ps[:, 0, :, :], t_bl[:, G_CYC:G_CYC + 16],
                             t_ptb[:], start=True, stop=True)
            nc.tensor.matmul(qq_ps[:, 1, :, :], t_bl[:, G_CYS:G_CYS + 16],
                             t_ptb[:], start=True, stop=True)
            t_qq = wpool.tile([16, 2, SPC, W], f32)
            nc.vector.tensor_copy(t_qq[:], qq_ps[:])

            # complex assembly (transposed layout [ky, r]), batched over samples
            t_qt = wpool.tile([16, SPC, 2, NR], f32)
            nc.vector.tensor_add(t_qt[:, :, 0, :],
                                 t_qq[:, 0, :, 0:NR], t_qq[:, 1, :, NR:W])
            nc.vector.tensor_sub(t_qt[:, :, 1, :],
                                 t_qq[:, 0, :, NR:W], t_qq[:, 1, :, 0:NR])

            # spectral multiplier: W'r = Sr*Qr - Si*Qi ; W'i = Sr*Qi + Si*Qr
            srt = t_bl[0:16, G_SRT:G_SRT + W * SPC].rearrange(
                "k (s b r) -> k s b r", s=SPC, b=2)
            sit = t_bl[0:16, G_SIT:G_SIT + W * SPC].rearrange(
                "k (s b r) -> k s b r", s=SPC, b=2)
            t_m1 = wpool.tile([16, SPC, 2, NR], f32)
            t_m2 = wpool.tile([16, SPC, 2, NR], f32)
            nc.vector.tensor_mul(t_m1[:], srt, t_qt[:])
            nc.vector.tensor_mul(t_m2[:], sit, t_qt[:])
            t_l1 = wpool.tile([16, SPC, 2, NR], f32)
            t_l2 = wpool.tile([16, SPC, 2, NR], f32)
            nc.vector.tensor_sub(t_l1[:, :, 0, :], t_m1[:, :, 0, :], t_m2[:, :, 1, :])
            nc.vector.tensor_add(t_l1[:, :, 1, :], t_m1[:, :, 1, :], t_m2[:, :, 0, :])
            nc.vector.tensor_scalar_mul(t_l2[:, :, 0, :], t_l1[:, :, 1, :], -1.0)
            nc.vector.tensor_copy(t_l2[:, :, 1, :], t_l1[:, :, 0, :])

            # y-inverse: V = [Vr; Vi] = L1.T @ CEy + L2.T @ SEy
            v_ps = ppool.tile([W, SPC, D], f32)
            for s in range(SPC):
                nc.tensor.matmul(v_ps[:, s, :], t_l1[:, s, :, :],
                                 t_bl[0:16, G_CEY:G_CEY + D], start=True, stop=False)
                nc.tensor.matmul(v_ps[:, s, :], t_l2[:, s, :, :],
                                 t_bl[0:16, G_SEY:G_SEY + D], start=False, stop=True)
            t_v = wpool.tile([W + 1, SPC, D], f32)
            # V row 66 = c_total; paired with the ones row 66 of the extended
            # x-inverse matrix this adds the scalar bias inside the matmul.
            # Engine partition bases must be 32-aligned, so copy rows 64:67
            # from the blob first and let the V copy overwrite rows 64:65.
            nc.vector.tensor_copy(t_v[64:W + 1, :, :],
                                  t_bl[64:W + 1, G_CROW:G_CROW + SPC * D])
            nc.vector.tensor_copy(t_v[0:W, :, :], v_ps[:])

            # x-inverse (incl. bias row), then out = s_base * z + O fused
            o_ps = ppool.tile([D, SPC, D], f32)
            for s in range(SPC):
                nc.tensor.matmul(o_ps[:, s, :], t_bl[0:W + 1, G_ICIS:G_ICIS + D],
                                 t_v[:, s, :], start=True, stop=True)
            t_out = wpool.tile([D, SPC, D], f32)
            nc.vector.scalar_tensor_tensor(
                t_out[:], t_z[:], t_bl[:, G_SB:G_SB + 1], o_ps[:],
                op0=ALU.mult, op1=ALU.add)

            nc.sync.dma_start(outc.rearrange("s x y -> x s y"), t_out[:])

    nc.finalize()
    return nc


def _run_device(z, consts):
    global LAST_RESULT
    import os
    from concourse.bass_utils import run_bass_kernel_spmd

    raw = os.environ.get("FNDE_IMPL", "raw") == "raw"
    key = "raw" if raw else "tile"
    if key not in _NC_CACHE:
        _NC_CACHE[key] = _build_nc_raw() if raw else _build_nc()
    nc = _NC_CACHE[key]
    in_maps = _shard_inputs_raw(z, consts) if raw else _shard_inputs(z, consts)
    res = run_bass_kernel_spmd(nc, in_maps, core_ids=list(range(NCORES)))
    LAST_RESULT = res
    out = np.empty((B, 1, D, D), np.float32)
    for i in range(NCORES):
        out[i * SPC:(i + 1) * SPC, 0] = res.results[i]["outc"]
    return out


def kernel(z, lift_w, lift_b, spec_w1, spec_w2, fl_bias, p1_w, p1_b, p2_w, p2_b,
           samp_ts):
    inputs = dict(z=z, lift_w=lift_w, lift_b=lift_b, spec_w1=spec_w1,
                  spec_w2=spec_w2, fl_bias=fl_bias, p1_w=p1_w, p1_b=p1_b,
                  p2_w=p2_w, p2_b=p2_b, samp_ts=samp_ts)
    consts = _fold_weights(inputs)
    z = np.asarray(z, np.float32)
    return _run_device(z, consts)


def kernel_numpy(z, **kw):
    """Pure-numpy path running the same folded math (validation only)."""
    inputs = dict(z=z, **kw)
    consts = _fold_weights(inputs)
    z = np.asarray(z, np.float32)
    out = np.empty((B, 1, D, D), np.float32)
    for i in range(NCORES):
        out[i * SPC:(i + 1) * SPC, 0] = _device_sim(z[i * SPC:(i + 1) * SPC, 0], consts)
    return out
